# revision 14
# baseline (speedup 1.0000x reference)
"""AttHierarchicalGround Trainium2 kernel.

Pure data parallel over batch (B=8 -> 1 element per NeuronCore). Host
pre-transposes the video tensor per core to [K=2176(pad), M=4800] bf16 so the
contraction dim lands on SBUF partitions with no on-device transposes of X.
All weights ship packed in one bf16 DRAM tensor (single DMA -> single sem
lane), biases in one f32 tensor. Matmuls run bf16 with fp32 PSUM accumulation;
elementwise/softmax/LN/LSTM state stay fp32.
"""

import os
import sys

import numpy as np

sys.path.insert(0, "/opt/trn_rl_repo")

import ml_dtypes  # noqa: E402

import concourse.bass as bass  # noqa: E402
import concourse.bacc as bacc  # noqa: E402
import concourse.mybir as mybir  # noqa: E402
import concourse.tile as tile  # noqa: E402
from concourse.masks import make_identity  # noqa: E402

BF16 = ml_dtypes.bfloat16
F32 = mybir.dt.float32
BF = mybir.dt.bfloat16
AF = mybir.ActivationFunctionType
ALU = mybir.AluOpType
AX = mybir.AxisListType

B, T, NB, VD, WD = 8, 120, 40, 2053, 300
H, E, NH, HD = 512, 256, 8, 64
SEG, NSEG = 12, 10
M = T * NB
KT = 17
CH = 480
NCH = M // CH

W512 = ["wtv", "wq", "wk", "wv", "wo", "wf1", "wf2", "wtr",
        "wtb1a", "wtb1b", "wl1a", "wl1b"]
# packed bf16 weight columns
WOFF = {"wev": (0, 4096), "wel": (4096, 256), "wew": (4352, 768),
        "wsp1a": (5120, 512), "wsp1b": (5632, 512)}
_off = 6144
for _nm in W512:
    WOFF[_nm] = (_off, 2048)
    _off += 2048
WOFF["wsp2"] = (_off, 2); _off += 2
WOFF["wtb2"] = (_off, 4); _off += 4
WOFF["wl2"] = (_off, 4); _off += 4
WCOLS = _off  # 30730

BNAMES2 = ["b_ev", "b_el", "b_ew", "b_sp1", "bmsg0", "bmsg1"]
BNAMES4 = ["b_tv", "bq", "bk", "bv", "bo", "bf1", "bf2", "b_tr",
           "b_tb1", "b_l1", "g1", "be1", "g2", "be2"]
BOFF = {}
_bo = 0
for _nm in BNAMES2:
    BOFF[_nm] = (_bo, 2)
    _bo += 2
for _nm in BNAMES4:
    BOFF[_nm] = (_bo, 4)
    _bo += 4
BOFF["bsum"] = (_bo, 16)
BCOLS = _bo + 16  # 84

_CACHE = {}
last_exec_time_ns = None


def _build_nc():
    nc = bacc.Bacc(trn_type="TRN2", target_bir_lowering=False)
    t = {}
    t["xt"] = nc.dram_tensor("xt", [KT, 128, M], BF, kind="ExternalInput")
    t["wpack"] = nc.dram_tensor("wpack", [128, WCOLS], BF, kind="ExternalInput")
    t["bpack"] = nc.dram_tensor("bpack", [128, BCOLS], F32, kind="ExternalInput")
    t["gpack"] = nc.dram_tensor("gpack", [128, 6], BF, kind="ExternalInput")
    t["wmsg0"] = nc.dram_tensor("wmsg0", [NB, E], BF, kind="ExternalInput")
    t["wmsg1"] = nc.dram_tensor("wmsg1", [NB, E], BF, kind="ExternalInput")
    t["wiht"] = nc.dram_tensor("wiht", [4, 128, 4 * H], BF, kind="ExternalInput")
    t["whht"] = nc.dram_tensor("whht", [4, 128, 4 * H], BF, kind="ExternalInput")
    t["o_out"] = nc.dram_tensor("o_out", [128, 4], F32, kind="ExternalOutput")
    t["o_ht"] = nc.dram_tensor("o_ht", [128, 4], F32, kind="ExternalOutput")
    t["o_ct"] = nc.dram_tensor("o_ct", [128, 4], F32, kind="ExternalOutput")
    t["o_suba"] = nc.dram_tensor("o_suba", [T, NB], F32, kind="ExternalOutput")
    t["o_obja"] = nc.dram_tensor("o_obja", [T, NB], F32, kind="ExternalOutput")
    t["o_beta1"] = nc.dram_tensor("o_beta1", [T], F32, kind="ExternalOutput")
    t["o_beta2"] = nc.dram_tensor("o_beta2", [NSEG], F32, kind="ExternalOutput")
    t["arow_dram"] = nc.dram_tensor("arow_scratch", [2, M], F32)
    with tile.TileContext(nc) as tc:
        _emit(nc, tc, t)
    nc.compile()
    return nc


def _emit(nc, tc, t):
    from contextlib import ExitStack

    ctx = ExitStack()
    with ctx:
        const = ctx.enter_context(tc.tile_pool(name="const", bufs=1))
        big = ctx.enter_context(tc.tile_pool(name="big", bufs=1))
        work = ctx.enter_context(tc.tile_pool(name="work", bufs=2))
        small = ctx.enter_context(tc.tile_pool(name="small", bufs=1))
        ps = ctx.enter_context(tc.tile_pool(name="ps", bufs=4, space="PSUM"))
        ps2 = ctx.enter_context(tc.tile_pool(name="ps2", bufs=4, space="PSUM"))

        ident = const.tile([128, 128], BF, tag="ident")
        make_identity(nc, ident)
        identf = const.tile([128, 128], F32, tag="identf")
        make_identity(nc, identf)

        wpack = const.tile([128, WCOLS], BF, tag="wpack")
        nc.sync.dma_start(out=wpack[:, :], in_=t["wpack"][:, :])
        bpack = const.tile([128, BCOLS], F32, tag="bpack")
        nc.sync.dma_start(out=bpack[:, :], in_=t["bpack"][:, :])
        gpack = const.tile([128, 6], BF, tag="gpack")
        nc.sync.dma_start(out=gpack[:, :], in_=t["gpack"][:, :])
        wmsg_sb = []
        for wi in range(2):
            mtl = const.tile([NB, E], BF, tag=f"wmsg{wi}")
            nc.sync.dma_start(out=mtl[:, :], in_=t[f"wmsg{wi}"][:, :])
            wmsg_sb.append(mtl)
        epst = const.tile([T, 1], F32, tag="epst")
        nc.vector.memset(epst[:, :], 1e-5)

        def wv(name, nk=None):
            off, n = WOFF[name]
            v = wpack[:, off:off + n]
            if nk is not None:
                v = v.rearrange("p (k e) -> p k e", k=nk)
            return v

        def bv_(name):
            off, n = BOFF[name]
            return bpack[:, off:off + n]

        wev_sb = wv("wev", 16)
        wel_sb = wv("wel")
        wew_sb = wv("wew", 3)
        wsp1a_sb = wv("wsp1a", 2)
        wsp1b_sb = wv("wsp1b", 2)
        wsp2_sb = wv("wsp2")
        w_sb = {nm: wv(nm, 4) for nm in W512}
        wtb2_sb = wv("wtb2")
        wl2_sb = wv("wl2")
        bias = {nm: bv_(nm) for nm in BNAMES2 + BNAMES4 + ["bsum"]}
        bmsg_b = [bias["bmsg0"], bias["bmsg1"]]

        # ---- word embeddings ----
        word_bf = []
        bvec = []
        for wi in range(2):
            gv = gpack[:, wi * 3:(wi + 1) * 3]
            wt = small.tile([128, 2], F32, tag=f"word_f{wi}")
            for eh in range(2):
                p = ps2.tile([128, 1], F32, tag="ps2")
                for kt in range(3):
                    nc.tensor.matmul(p[:, :], wew_sb[:, kt, eh * 128:(eh + 1) * 128],
                                     gv[:, kt:kt + 1], start=(kt == 0), stop=(kt == 2))
                nc.scalar.activation(wt[:, eh:eh + 1], p[:, :], AF.Relu,
                                     bias=bias["b_ew"][:, eh:eh + 1])
            wbf = small.tile([128, 2], BF, tag=f"word_bf{wi}")
            nc.vector.tensor_copy(wbf[:, :], wt[:, :])
            word_bf.append(wbf)
            bv = small.tile([128, 2], F32, tag=f"bvec{wi}")
            for eh in range(2):
                p = ps2.tile([128, 1], F32, tag="ps2")
                for kt in range(2):
                    nc.tensor.matmul(p[:, :], wsp1b_sb[:, kt, eh * 128:(eh + 1) * 128],
                                     wbf[:, kt:kt + 1], start=(kt == 0), stop=(kt == 1))
                nc.vector.scalar_tensor_tensor(
                    bv[:, eh:eh + 1], p[:, :], 1.0, bias["b_sp1"][:, eh:eh + 1],
                    op0=ALU.mult, op1=ALU.add)
            bvec.append(bv)
        relT_bf = small.tile([128, 4], BF, tag="relT")
        nc.vector.tensor_copy(relT_bf[:, 0:2], word_bf[0][:, :])
        nc.vector.tensor_copy(relT_bf[:, 2:4], word_bf[1][:, :])

        # ---- phase A ----
        veT = big.tile([128, 2, M], BF, tag="veT")
        with tc.tile_pool(name="xtp", bufs=2) as xtp:
            for c in range(NCH):
                sl = slice(c * CH, (c + 1) * CH)
                xt_t = xtp.tile([128, KT, CH], BF, tag="xt")
                nc.sync.dma_start(out=xt_t[:, :, :],
                                  in_=t["xt"][:, :, sl].rearrange("k p m -> p k m"))
                for eh in range(2):
                    esl = slice(eh * 128, (eh + 1) * 128)
                    pA = ps.tile([128, CH], F32, tag="ps")
                    for kt in range(16):
                        nc.tensor.matmul(pA[:, :], wev_sb[:, kt, esl], xt_t[:, kt, :],
                                         start=(kt == 0), stop=(kt == 15))
                    pB = ps.tile([128, CH], F32, tag="ps")
                    nc.tensor.matmul(pB[:, :], wel_sb[:, esl], xt_t[:, 16, :],
                                     start=True, stop=True)
                    t1 = work.tile([128, CH], F32, tag="t1")
                    t2 = work.tile([128, CH], F32, tag="t2")
                    nc.scalar.activation(t1[:, :], pA[:, :], AF.Relu,
                                         bias=bias["b_ev"][:, eh:eh + 1])
                    nc.scalar.activation(t2[:, :], pB[:, :], AF.Relu,
                                         bias=bias["b_el"][:, eh:eh + 1])
                    nc.vector.tensor_add(veT[:, eh, sl], t1[:, :], t2[:, :])

        lstmw = ctx.enter_context(tc.tile_pool(name="lstmw", bufs=1))
        wiht_sb = lstmw.tile([128, 4, 4 * H], BF, tag="wiht")
        nc.sync.dma_start(out=wiht_sb[:, :, :],
                          in_=t["wiht"][:, :, :].rearrange("k p g -> p k g"))
        whht_sb = lstmw.tile([128, 4, 4 * H], BF, tag="whht")
        nc.sync.dma_start(out=whht_sb[:, :, :],
                          in_=t["whht"][:, :, :].rearrange("k p g -> p k g"))

        # ---- phase B: spatial attention ----
        featT = []
        msgT = []
        att_out = [t["o_suba"], t["o_obja"]]
        for wi in range(2):
            srow = big.tile([1, M], F32, tag=f"srow{wi}")
            for c in range(NCH):
                sl = slice(c * CH, (c + 1) * CH)
                tanhP = work.tile([128, 2, CH], BF, tag="tanhP")
                for eh in range(2):
                    esl = slice(eh * 128, (eh + 1) * 128)
                    pP = ps.tile([128, CH], F32, tag="ps")
                    for kt in range(2):
                        nc.tensor.matmul(pP[:, :], wsp1a_sb[:, kt, esl],
                                         veT[:, kt, sl], start=(kt == 0), stop=(kt == 1))
                    nc.scalar.activation(tanhP[:, eh, :], pP[:, :], AF.Tanh,
                                         bias=bvec[wi][:, eh:eh + 1])
                pS = ps2.tile([1, CH], F32, tag="ps2")
                for kt in range(2):
                    nc.tensor.matmul(pS[:, :], wsp2_sb[:, kt:kt + 1],
                                     tanhP[:, kt, :], start=(kt == 0), stop=(kt == 1))
                nc.scalar.copy(srow[:, sl], pS[:, :])
            S = small.tile([T, NB], F32, tag="S")
            nc.sync.dma_start(out=S[:, :],
                              in_=srow.rearrange("p (t n) -> p t n", t=T))
            mx = small.tile([T, 1], F32, tag="mx")
            nc.vector.tensor_reduce(out=mx[:, :], in_=S[:, :], axis=AX.X, op=ALU.max)
            nmx = small.tile([T, 1], F32, tag="nmx")
            nc.vector.tensor_scalar_mul(nmx[:, :], mx[:, :], -1.0)
            Ea = small.tile([T, NB], F32, tag="Ea")
            nc.scalar.activation(Ea[:, :], S[:, :], AF.Exp, bias=nmx[:, :])
            sm = small.tile([T, 1], F32, tag="sm")
            nc.vector.tensor_reduce(out=sm[:, :], in_=Ea[:, :], axis=AX.X, op=ALU.add)
            rc = small.tile([T, 1], F32, tag="rc")
            nc.vector.reciprocal(rc[:, :], sm[:, :])
            a_f = small.tile([T, NB], F32, tag="a_f")
            nc.vector.tensor_scalar_mul(a_f[:, :], Ea[:, :], rc[:, :])
            nc.sync.dma_start(out=att_out[wi][:, :], in_=a_f[:, :])
            nc.sync.dma_start(out=t["arow_dram"][wi, :], in_=a_f[:, :])
            a_bf = small.tile([T, NB], BF, tag="a_bf")
            nc.vector.tensor_copy(a_bf[:, :], a_f[:, :])
            paT = ps2.tile([NB, T], BF, tag="ps2")
            nc.tensor.transpose(paT[:, :], a_bf[:, :], ident[:T, :T])
            aT_bf = small.tile([NB, T], BF, tag="aT_bf")
            nc.scalar.copy(aT_bf[:, :], paT[:, :])
            mg = small.tile([128, 2, T], F32, tag=f"msgT{wi}")
            for eh in range(2):
                pm = ps.tile([128, T], F32, tag="ps")
                nc.tensor.matmul(pm[:, :], wmsg_sb[wi][:, eh * 128:(eh + 1) * 128],
                                 aT_bf[:, :], start=True, stop=True)
                nc.scalar.activation(mg[:, eh, :], pm[:, :], AF.Relu,
                                     bias=bmsg_b[wi][:, eh:eh + 1])
            msgT.append(mg)
            ft = small.tile([128, 2, T], F32, tag=f"featT{wi}")
            for c in range(NCH):
                aB = work.tile([128, CH], BF, tag="aB")
                nc.gpsimd.dma_start(out=aB[:, :], in_=bass.AP(
                    tensor=t["arow_dram"][:, :].tensor, offset=wi * M + c * CH,
                    ap=[[0, 128], [1, CH]]))
                for eh in range(2):
                    prod = work.tile([128, SEG, NB], F32, tag="prod")
                    nc.vector.tensor_mul(
                        prod[:, :, :],
                        veT[:, eh, c * CH:(c + 1) * CH].rearrange(
                            "p (s n) -> p s n", s=SEG),
                        aB.rearrange("p (s n) -> p s n", s=SEG))
                    nc.vector.tensor_reduce(
                        out=ft[:, eh, c * SEG:(c + 1) * SEG], in_=prod[:, :, :],
                        axis=AX.X, op=ALU.add)
            featT.append(ft)

        # ---- transformer ----
        ori_bf = small.tile([128, 4, T], BF, tag="ori_bf")
        nc.vector.tensor_add(ori_bf[:, 0:2, :], featT[0][:, :, :], msgT[1][:, :, :])
        nc.vector.tensor_add(ori_bf[:, 2:4, :], featT[1][:, :, :], msgT[0][:, :, :])

        def mm512(wname, rhs_bf, n):
            outs = []
            for mh in range(4):
                p = ps.tile([128, n], F32, tag="ps")
                for kt in range(4):
                    nc.tensor.matmul(p[:, :],
                                     w_sb[wname][:, kt, mh * 128:(mh + 1) * 128],
                                     rhs_bf[:, kt, :], start=(kt == 0), stop=(kt == 3))
                outs.append(p)
            return outs

        xT_f = small.tile([128, 4, T], F32, tag="xT_f")
        xT_bf = small.tile([128, 4, T], BF, tag="xT_bf")
        for mh, p in enumerate(mm512("wtv", ori_bf, T)):
            nc.scalar.activation(xT_f[:, mh, :], p[:, :], AF.Relu,
                                 bias=bias["b_tv"][:, mh:mh + 1])
            nc.vector.tensor_copy(xT_bf[:, mh, :], xT_f[:, mh, :])

        qkv = {}
        for nm, wn, bn in [("q", "wq", "bq"), ("k", "wk", "bk"), ("v", "wv", "bv")]:
            dst = small.tile([128, 4, T], BF, tag=nm + "T")
            for mh, p in enumerate(mm512(wn, xT_bf, T)):
                nc.vector.tensor_scalar_add(dst[:, mh, :], p[:, :],
                                            bias[bn][:, mh:mh + 1])
            qkv[nm] = dst

        aoT_bf = small.tile([128, 4, T], BF, tag="aoT")
        for h in range(NH):
            hh, off = h // 2, 64 * (h % 2)
            pSc = ps2.tile([T, T], F32, tag="ps2")
            nc.tensor.matmul(pSc[:, :], qkv["q"][off:off + 64, hh, :],
                             qkv["k"][off:off + 64, hh, :], start=True, stop=True)
            mx1 = small.tile([T, 1], F32, tag="mx1")
            nc.vector.tensor_reduce(out=mx1[:, :], in_=pSc[:, :], axis=AX.X, op=ALU.max)
            nmx1 = small.tile([T, 1], F32, tag="nmx1")
            nc.vector.tensor_scalar_mul(nmx1[:, :], mx1[:, :], -0.125)
            Eh = small.tile([T, T], F32, tag="Eh")
            nc.scalar.activation(Eh[:, :], pSc[:, :], AF.Exp,
                                 bias=nmx1[:, :], scale=0.125)
            sm1 = small.tile([T, 1], F32, tag="sm1")
            nc.vector.tensor_reduce(out=sm1[:, :], in_=Eh[:, :], axis=AX.X, op=ALU.add)
            rc1 = small.tile([T, 1], F32, tag="rc1")
            nc.vector.reciprocal(rc1[:, :], sm1[:, :])
            Ehn = small.tile([T, T], BF, tag="Ehn")
            nc.vector.tensor_scalar_mul(Ehn[:, :], Eh[:, :], rc1[:, :])
            pET = ps2.tile([T, T], BF, tag="ps2")
            nc.tensor.transpose(pET[:, :], Ehn[:, :], ident[:T, :T])
            EhT = small.tile([T, T], BF, tag="EhT")
            nc.scalar.copy(EhT[:, :], pET[:, :])
            pvn = ps2.tile([T, 64], BF, tag="ps2")
            nc.tensor.transpose(pvn[:, :], qkv["v"][off:off + 64, hh, :],
                                ident[off:off + 64, off:off + 64])
            vnat = small.tile([T, 64], BF, tag="vnat")
            nc.scalar.copy(vnat[:, :], pvn[:, :])
            pao = ps2.tile([64, T], F32, tag="ps2")
            nc.tensor.matmul(pao[:, :], vnat[:, :], EhT[:, :], start=True, stop=True)
            nc.scalar.copy(aoT_bf[off:off + 64, hh, :], pao[:, :])

        x1 = small.tile([128, 4, T], F32, tag="x1")
        for mh, p in enumerate(mm512("wo", aoT_bf, T)):
            tt = work.tile([128, T], F32, tag="ott")
            nc.vector.tensor_scalar_add(tt[:, :], p[:, :], bias["bo"][:, mh:mh + 1])
            nc.vector.tensor_add(x1[:, mh, :], xT_f[:, mh, :], tt[:, :])

        def layer_norm(src_f, gname, bname, out_f, out_bf, also_nat_bf=None):
            xnat = small.tile([T, 4, 128], F32, tag="xnat")
            for mh in range(4):
                pt = ps2.tile([T, 128], F32, tag="ps2")
                nc.tensor.transpose(pt[:, :], src_f[:, mh, :], identf[:, :])
                nc.scalar.copy(xnat[:, mh, :], pt[:, :])
            stats = small.tile([T, 6], F32, tag="stats")
            nc.vector.bn_stats(out=stats[:, :], in_=xnat.rearrange("t a b -> t (a b)"))
            mv = small.tile([T, 2], F32, tag="mv")
            nc.vector.bn_aggr(out=mv[:, :], in_=stats[:, :])
            sq = small.tile([T, 1], F32, tag="sq")
            nc.scalar.activation(sq[:, :], mv[:, 1:2], AF.Sqrt, bias=epst[:, :])
            rstd = small.tile([T, 1], F32, tag="rstd")
            nc.vector.reciprocal(rstd[:, :], sq[:, :])
            nmean = small.tile([T, 1], F32, tag="nmean")
            nc.vector.tensor_scalar_mul(nmean[:, :], mv[:, 0:1], -1.0)
            xn = small.tile([T, 4, 128], F32, tag="xn")
            nc.vector.tensor_scalar(xn[:, :, :], xnat[:, :, :], nmean[:, :],
                                    rstd[:, :], op0=ALU.add, op1=ALU.mult)
            if also_nat_bf is not None:
                nc.vector.tensor_copy(also_nat_bf[:, :, :], xn[:, :, :])
            for mh in range(4):
                pt = ps2.tile([128, T], F32, tag="ps2")
                nc.tensor.transpose(pt[:, :], xn[:, mh, :], identf[:T, :T])
                nc.vector.tensor_scalar(out_f[:, mh, :], pt[:, :],
                                        bias[gname][:, mh:mh + 1],
                                        bias[bname][:, mh:mh + 1],
                                        op0=ALU.mult, op1=ALU.add)
                nc.vector.tensor_copy(out_bf[:, mh, :], out_f[:, mh, :])

        x2 = small.tile([128, 4, T], F32, tag="x2")
        x2_bf = small.tile([128, 4, T], BF, tag="x2bf")
        layer_norm(x1, "g1", "be1", x2, x2_bf)

        f1_bf = small.tile([128, 4, T], BF, tag="f1bf")
        for mh, p in enumerate(mm512("wf1", x2_bf, T)):
            nc.scalar.activation(f1_bf[:, mh, :], p[:, :], AF.Relu,
                                 bias=bias["bf1"][:, mh:mh + 1])
        x3 = small.tile([128, 4, T], F32, tag="x3")
        for mh, p in enumerate(mm512("wf2", f1_bf, T)):
            tt = work.tile([128, T], F32, tag="ftt")
            nc.vector.tensor_scalar_add(tt[:, :], p[:, :], bias["bf2"][:, mh:mh + 1])
            nc.vector.tensor_add(x3[:, mh, :], x2[:, mh, :], tt[:, :])

        withinT = small.tile([128, 4, T], F32, tag="withinT")
        withinT_bf = small.tile([128, 4, T], BF, tag="withinTbf")
        within_nat_bf = small.tile([T, 4, 128], BF, tag="withinnat")
        layer_norm(x3, "g2", "be2", withinT, withinT_bf,
                   also_nat_bf=within_nat_bf)

        # ---- phase D ----
        seg_bf = withinT_bf[:, :, SEG - 1::SEG]

        trT_bf = small.tile([128, 4], BF, tag="trTbf")
        for mh in range(4):
            p = ps2.tile([128, 1], F32, tag="ps2")
            for kt in range(4):
                nc.tensor.matmul(p[:, :], w_sb["wtr"][:, kt, mh * 128:(mh + 1) * 128],
                                 relT_bf[:, kt:kt + 1], start=(kt == 0), stop=(kt == 3))
            nc.scalar.activation(trT_bf[:, mh:mh + 1], p[:, :], AF.Relu,
                                 bias=bias["b_tr"][:, mh:mh + 1])

        ub2 = small.tile([128, 4], F32, tag="ub2")
        for mh in range(4):
            p = ps2.tile([128, 1], F32, tag="ps2")
            for kt in range(4):
                nc.tensor.matmul(p[:, :], w_sb["wtb1b"][:, kt, mh * 128:(mh + 1) * 128],
                                 trT_bf[:, kt:kt + 1], start=(kt == 0), stop=(kt == 3))
            nc.vector.tensor_scalar_add(ub2[:, mh:mh + 1], p[:, :],
                                        bias["b_tb1"][:, mh:mh + 1])

        z2_bf = small.tile([128, 4, NSEG], BF, tag="z2bf")
        for mh in range(4):
            p = ps2.tile([128, NSEG], F32, tag="ps2")
            for kt in range(4):
                nc.tensor.matmul(p[:, :], w_sb["wtb1a"][:, kt, mh * 128:(mh + 1) * 128],
                                 seg_bf[:, kt, :], start=(kt == 0), stop=(kt == 3))
            nc.scalar.activation(z2_bf[:, mh, :], p[:, :], AF.Tanh,
                                 bias=ub2[:, mh:mh + 1])

        def row_softmax(psrc, n, tag):
            row = small.tile([1, n], F32, tag=tag + "r")
            nc.scalar.copy(row[:, :], psrc[:, :])
            m_ = small.tile([1, 1], F32, tag=tag + "m")
            nc.vector.tensor_reduce(out=m_[:, :], in_=row[:, :], axis=AX.X, op=ALU.max)
            nm_ = small.tile([1, 1], F32, tag=tag + "nm")
            nc.vector.tensor_scalar_mul(nm_[:, :], m_[:, :], -1.0)
            e_ = small.tile([1, n], F32, tag=tag + "e")
            nc.scalar.activation(e_[:, :], row[:, :], AF.Exp, bias=nm_[:, :])
            s_ = small.tile([1, 1], F32, tag=tag + "s")
            nc.vector.tensor_reduce(out=s_[:, :], in_=e_[:, :], axis=AX.X, op=ALU.add)
            r_ = small.tile([1, 1], F32, tag=tag + "rc")
            nc.vector.reciprocal(r_[:, :], s_[:, :])
            o_ = small.tile([1, n], F32, tag=tag + "o")
            nc.vector.tensor_scalar_mul(o_[:, :], e_[:, :], r_[:, :])
            return o_

        pb2 = ps2.tile([1, NSEG], F32, tag="ps2")
        for kt in range(4):
            nc.tensor.matmul(pb2[:, :], wtb2_sb[:, kt:kt + 1], z2_bf[:, kt, :],
                             start=(kt == 0), stop=(kt == 3))
        beta2row = row_softmax(pb2, NSEG, "b2")
        nc.sync.dma_start(out=t["o_beta2"][None, :], in_=beta2row[:, :])

        XWT = small.tile([128, 16, NSEG], F32, tag="XWT")
        for gt in range(16):
            p = ps2.tile([128, NSEG], F32, tag="ps2")
            for kt in range(4):
                nc.tensor.matmul(p[:, :], wiht_sb[:, kt, gt * 128:(gt + 1) * 128],
                                 seg_bf[:, kt, :], start=(kt == 0), stop=(kt == 3))
            nc.vector.tensor_scalar_add(XWT[:, gt, :], p[:, :],
                                        bias["bsum"][:, gt:gt + 1])

        lstm = ctx.enter_context(tc.tile_pool(name="lstm", bufs=3))
        h_bf = lstm.tile([128, 4], BF, tag="h_bf")
        c_f = lstm.tile([128, 4], F32, tag="c_f")
        nc.vector.memset(h_bf[:, :], 0.0)
        nc.vector.memset(c_f[:, :], 0.0)
        h_f = None
        for s in range(NSEG):
            pg = ps.tile([128, 16], F32, tag="ps")
            for gt in range(16):
                for kt in range(4):
                    nc.tensor.matmul(pg[:, gt:gt + 1],
                                     whht_sb[:, kt, gt * 128:(gt + 1) * 128],
                                     h_bf[:, kt:kt + 1],
                                     start=(kt == 0), stop=(kt == 3))
            G = lstm.tile([128, 16], F32, tag="G")
            nc.vector.tensor_add(G[:, :], pg[:, :], XWT[:, :, s])
            SGt = lstm.tile([128, 16], F32, tag="SGt")
            nc.scalar.activation(SGt[:, 0:8], G[:, 0:8], AF.Sigmoid)
            nc.scalar.activation(SGt[:, 8:12], G[:, 8:12], AF.Tanh)
            nc.scalar.activation(SGt[:, 12:16], G[:, 12:16], AF.Sigmoid)
            t1 = lstm.tile([128, 4], F32, tag="lt1")
            t2 = lstm.tile([128, 4], F32, tag="lt2")
            nc.vector.tensor_mul(t1[:, :], SGt[:, 4:8], c_f[:, :])
            nc.vector.tensor_mul(t2[:, :], SGt[:, 0:4], SGt[:, 8:12])
            c_f = lstm.tile([128, 4], F32, tag="c_f")
            nc.vector.tensor_add(c_f[:, :], t1[:, :], t2[:, :])
            tct = lstm.tile([128, 4], F32, tag="tct")
            nc.scalar.activation(tct[:, :], c_f[:, :], AF.Tanh)
            h_f = lstm.tile([128, 4], F32, tag="h_f")
            nc.vector.tensor_mul(h_f[:, :], SGt[:, 12:16], tct[:, :])
            h_bf = lstm.tile([128, 4], BF, tag="h_bf")
            nc.vector.tensor_copy(h_bf[:, :], h_f[:, :])
        nc.sync.dma_start(out=t["o_ht"][:, :], in_=h_f[:, :])
        nc.sync.dma_start(out=t["o_ct"][:, :], in_=c_f[:, :])

        ub1 = small.tile([128, 4], F32, tag="ub1")
        for mh in range(4):
            p = ps2.tile([128, 1], F32, tag="ps2")
            for kt in range(4):
                nc.tensor.matmul(p[:, :], w_sb["wl1b"][:, kt, mh * 128:(mh + 1) * 128],
                                 h_bf[:, kt:kt + 1], start=(kt == 0), stop=(kt == 3))
            nc.vector.tensor_scalar_add(ub1[:, mh:mh + 1], p[:, :],
                                        bias["b_l1"][:, mh:mh + 1])
        z1_bf = small.tile([128, 4, T], BF, tag="z1bf")
        for mh in range(4):
            p = ps.tile([128, T], F32, tag="ps")
            for kt in range(4):
                nc.tensor.matmul(p[:, :], w_sb["wl1a"][:, kt, mh * 128:(mh + 1) * 128],
                                 withinT_bf[:, kt, :], start=(kt == 0), stop=(kt == 3))
            nc.scalar.activation(z1_bf[:, mh, :], p[:, :], AF.Tanh,
                                 bias=ub1[:, mh:mh + 1])
        pb1 = ps2.tile([1, T], F32, tag="ps2")
        for kt in range(4):
            nc.tensor.matmul(pb1[:, :], wl2_sb[:, kt:kt + 1], z1_bf[:, kt, :],
                             start=(kt == 0), stop=(kt == 3))
        beta1row = row_softmax(pb1, T, "b1")
        nc.sync.dma_start(out=t["o_beta1"][None, :], in_=beta1row[:, :])

        temp_bf = small.tile([1, T], BF, tag="tempbf")
        b2rep = bass.AP(tensor=beta2row[:, :].tensor, offset=beta2row[:, :].offset,
                        ap=[list(beta2row[:, :].ap[0]), [1, NSEG], [0, SEG]])
        nc.vector.tensor_tensor(
            temp_bf[:, :],
            beta1row.rearrange("p (a b) -> p a b", a=NSEG), b2rep, op=ALU.add)
        ptT = ps2.tile([T, 1], BF, tag="ps2")
        nc.tensor.transpose(ptT[:, :], temp_bf[:, :], ident[:1, :1])
        tempT_bf = small.tile([T, 1], BF, tag="tempTbf")
        nc.scalar.copy(tempT_bf[:, :], ptT[:, :])
        outv = small.tile([128, 4], F32, tag="outv")
        for mh in range(4):
            p = ps2.tile([128, 1], F32, tag="ps2")
            nc.tensor.matmul(p[:, :], within_nat_bf[:, mh, :], tempT_bf[:, :],
                             start=True, stop=True)
            nc.scalar.copy(outv[:, mh:mh + 1], p[:, :])
        nc.sync.dma_start(out=t["o_out"][:, :], in_=outv[:, :])


def _prep_maps(inputs):
    f = {k: np.asarray(v, np.float32) for k, v in inputs.items()}

    def ktp(a, nt):  # [nt*128, n] -> [128, nt*n] partition-major pack
        return np.ascontiguousarray(
            a.reshape(nt, 128, a.shape[1]).transpose(1, 0, 2).reshape(128, -1))

    def pad_rows(a, rows):
        out = np.zeros((rows, a.shape[1]), a.dtype)
        out[: a.shape[0]] = a
        return out

    def pcol(v, nt):
        return np.ascontiguousarray(v.reshape(nt, 128).T)

    wp = np.zeros((128, WCOLS), np.float32)
    wp[:, 0:4096] = ktp(f["W_ev"], 16)
    wp[:, 4096:4352] = pad_rows(f["W_el"], 128)
    wp[:, 4352:5120] = ktp(pad_rows(f["W_ew"], 384), 3)
    wp[:, 5120:5632] = ktp(f["W_sp1"][:E], 2)
    wp[:, 5632:6144] = ktp(f["W_sp1"][E:], 2)
    for nm, src in [("wtv", "W_tv"), ("wq", "Wq"), ("wk", "Wk"), ("wv", "Wv"),
                    ("wo", "Wo"), ("wf1", "Wf1"), ("wf2", "Wf2"),
                    ("wtr", "W_tr")]:
        off, n = WOFF[nm]
        wp[:, off:off + n] = ktp(f[src], 4)
    for nm, arr in [("wtb1a", f["W_tb1"][:H]), ("wtb1b", f["W_tb1"][H:]),
                    ("wl1a", f["W_l1"][:H]), ("wl1b", f["W_l1"][H:])]:
        off, n = WOFF[nm]
        wp[:, off:off + n] = ktp(arr, 4)
    for nm, v, nt in [("wsp2", f["w_sp2"], 2), ("wtb2", f["w_tb2"], 4),
                      ("wl2", f["w_l2"], 4)]:
        off, n = WOFF[nm]
        wp[:, off:off + n] = pcol(v, nt)
    wpack = np.ascontiguousarray(wp.astype(BF16))

    bp = np.zeros((128, BCOLS), np.float32)
    for nm, src, nt in [("b_ev", "b_ev", 2), ("b_el", "b_el", 2),
                        ("b_ew", "b_ew", 2), ("b_sp1", "b_sp1", 2),
                        ("bmsg0", "b_s2o", 2), ("bmsg1", "b_o2s", 2),
                        ("b_tv", "b_tv", 4), ("bq", "bq", 4), ("bk", "bk", 4),
                        ("bv", "bv", 4), ("bo", "bo", 4), ("bf1", "bf1", 4),
                        ("bf2", "bf2", 4), ("b_tr", "b_tr", 4),
                        ("b_tb1", "b_tb1", 4), ("b_l1", "b_l1", 4),
                        ("g1", "g1", 4), ("be1", "be1", 4), ("g2", "g2", 4),
                        ("be2", "be2", 4)]:
        off, n = BOFF[nm]
        bp[:, off:off + n] = pcol(f[src], nt)
    off, n = BOFF["bsum"]
    bp[:, off:off + n] = pcol(f["b_ih"] + f["b_hh"], 16)
    bpack = np.ascontiguousarray(bp)

    shared = {
        "wpack": wpack,
        "bpack": bpack,
        "wmsg0": np.ascontiguousarray(f["W_s2o"].astype(BF16)),
        "wmsg1": np.ascontiguousarray(f["W_o2s"].astype(BF16)),
        "wiht": np.ascontiguousarray(
            f["W_ih"].T.reshape(4, 128, 4 * H).astype(BF16)),
        "whht": np.ascontiguousarray(
            f["W_hh"].T.reshape(4, 128, 4 * H).astype(BF16)),
    }

    in_maps = []
    for b in range(B):
        m = dict(shared)
        x = f["videos"][b].reshape(M, VD)
        xt = np.zeros((KT * 128, M), BF16)
        xt[:VD] = np.ascontiguousarray(x.T).astype(BF16)
        m["xt"] = xt.reshape(KT, 128, M)
        gp = np.zeros((128, 6), np.float32)
        gp[:, 0:3] = pad_rows(f["sub_glove"][b][:, None], 384).reshape(3, 128).T
        gp[:, 3:6] = pad_rows(f["obj_glove"][b][:, None], 384).reshape(3, 128).T
        m["gpack"] = np.ascontiguousarray(gp.astype(BF16))
        in_maps.append(m)
    return in_maps


def kernel(**inputs):
    global last_exec_time_ns
    from concourse.bass_utils import run_bass_kernel_spmd

    if "nc" not in _CACHE:
        _CACHE["nc"] = _build_nc()
    nc = _CACHE["nc"]
    in_maps = _prep_maps(inputs)
    res = run_bass_kernel_spmd(nc, in_maps, list(range(B)), trace=False)
    outs = res.results
    if os.environ.get("KERNEL_TRACE") == "1":
        import time as _time

        best = None
        for _ in range(3):
            t0 = _time.perf_counter()
            run_bass_kernel_spmd(nc, in_maps, list(range(B)), trace=False)
            dt = _time.perf_counter() - t0
            best = dt if best is None else min(best, dt)
        last_exec_time_ns = int(best * 1e9)
    output = np.stack([r["o_out"].T.reshape(H) for r in outs]).astype(np.float32)
    hT = np.stack([r["o_ht"].T.reshape(H) for r in outs]).astype(np.float32)
    cT = np.stack([r["o_ct"].T.reshape(H) for r in outs]).astype(np.float32)
    sub_att = np.stack([r["o_suba"] for r in outs]).astype(np.float32)
    obj_att = np.stack([r["o_obja"] for r in outs]).astype(np.float32)
    beta1 = np.stack([r["o_beta1"] for r in outs]).astype(np.float32)
    beta2 = np.stack([r["o_beta2"] for r in outs]).astype(np.float32)
    return (output, hT, cT, sub_att, obj_att, beta1, beta2)


# revision 15
# speedup vs baseline: 7.2057x; 7.2057x over previous
"""AttHierarchicalGround Trainium2 kernel.

Pure data parallel over batch (B=8 -> 1 element per NeuronCore). Host
pre-transposes the video tensor per core to [K=2176(pad), M=4800] bf16 so the
contraction dim lands on SBUF partitions with no on-device transposes of X.
All weights ship packed in one bf16 DRAM tensor (single DMA -> single sem
lane), biases in one f32 tensor. Matmuls run bf16 with fp32 PSUM accumulation;
elementwise/softmax/LN/LSTM state stay fp32.
"""

import os
import sys

import numpy as np

sys.path.insert(0, "/opt/trn_rl_repo")

import ml_dtypes  # noqa: E402

import concourse.bass as bass  # noqa: E402
import concourse.bacc as bacc  # noqa: E402
import concourse.mybir as mybir  # noqa: E402
import concourse.tile as tile  # noqa: E402
from concourse.masks import make_identity  # noqa: E402

BF16 = ml_dtypes.bfloat16
F32 = mybir.dt.float32
BF = mybir.dt.bfloat16
AF = mybir.ActivationFunctionType
ALU = mybir.AluOpType
AX = mybir.AxisListType

B, T, NB, VD, WD = 8, 120, 40, 2053, 300
H, E, NH, HD = 512, 256, 8, 64
SEG, NSEG = 12, 10
M = T * NB
KT = 17
CH = 480
NCH = M // CH

W512 = ["wtv", "wq", "wk", "wv", "wo", "wf1", "wf2", "wtr",
        "wtb1a", "wtb1b", "wl1a", "wl1b"]
# packed bf16 weight columns
WOFF = {"wev": (0, 4096), "wel": (4096, 256), "wew": (4352, 768),
        "wsp1a": (5120, 512), "wsp1b": (5632, 512)}
_off = 6144
for _nm in W512:
    WOFF[_nm] = (_off, 2048)
    _off += 2048
WOFF["wsp2"] = (_off, 2); _off += 2
WOFF["wtb2"] = (_off, 4); _off += 4
WOFF["wl2"] = (_off, 4); _off += 4
WCOLS = _off  # 30730

BNAMES2 = ["b_ev", "b_el", "b_ew", "b_sp1", "bmsg0", "bmsg1"]
BNAMES4 = ["b_tv", "bq", "bk", "bv", "bo", "bf1", "bf2", "b_tr",
           "b_tb1", "b_l1", "g1", "be1", "g2", "be2"]
BOFF = {}
_bo = 0
for _nm in BNAMES2:
    BOFF[_nm] = (_bo, 2)
    _bo += 2
for _nm in BNAMES4:
    BOFF[_nm] = (_bo, 4)
    _bo += 4
BOFF["bsum"] = (_bo, 16)
BCOLS = _bo + 16  # 84

_CACHE = {}
last_exec_time_ns = None


def _build_nc():
    nc = bacc.Bacc(trn_type="TRN2", target_bir_lowering=False)
    t = {}
    t["xt"] = nc.dram_tensor("xt", [KT, 128, M], BF, kind="ExternalInput")
    t["wpack"] = nc.dram_tensor("wpack", [128, WCOLS], BF, kind="ExternalInput")
    t["bpack"] = nc.dram_tensor("bpack", [128, BCOLS], F32, kind="ExternalInput")
    t["gpack"] = nc.dram_tensor("gpack", [128, 6], BF, kind="ExternalInput")
    t["wmsg0"] = nc.dram_tensor("wmsg0", [NB, E], BF, kind="ExternalInput")
    t["wmsg1"] = nc.dram_tensor("wmsg1", [NB, E], BF, kind="ExternalInput")
    t["wiht"] = nc.dram_tensor("wiht", [4, 128, 4 * H], BF, kind="ExternalInput")
    t["whht"] = nc.dram_tensor("whht", [4, 128, 4 * H], BF, kind="ExternalInput")
    t["o_out"] = nc.dram_tensor("o_out", [128, 4], F32, kind="ExternalOutput")
    t["o_ht"] = nc.dram_tensor("o_ht", [128, 4], F32, kind="ExternalOutput")
    t["o_ct"] = nc.dram_tensor("o_ct", [128, 4], F32, kind="ExternalOutput")
    t["o_suba"] = nc.dram_tensor("o_suba", [T, NB], F32, kind="ExternalOutput")
    t["o_obja"] = nc.dram_tensor("o_obja", [T, NB], F32, kind="ExternalOutput")
    t["o_beta1"] = nc.dram_tensor("o_beta1", [T], F32, kind="ExternalOutput")
    t["o_beta2"] = nc.dram_tensor("o_beta2", [NSEG], F32, kind="ExternalOutput")
    t["arow_dram"] = nc.dram_tensor("arow_scratch", [2, M], F32)
    with tile.TileContext(nc) as tc:
        _emit(nc, tc, t)
    nc.compile()
    return nc


def _emit(nc, tc, t):
    from contextlib import ExitStack

    ctx = ExitStack()
    with ctx:
        const = ctx.enter_context(tc.tile_pool(name="const", bufs=1))
        big = ctx.enter_context(tc.tile_pool(name="big", bufs=1))
        work = ctx.enter_context(tc.tile_pool(name="work", bufs=2))
        small = ctx.enter_context(tc.tile_pool(name="small", bufs=1))
        ps = ctx.enter_context(tc.tile_pool(name="ps", bufs=4, space="PSUM"))
        ps2 = ctx.enter_context(tc.tile_pool(name="ps2", bufs=4, space="PSUM"))

        ident = const.tile([128, 128], BF, tag="ident")
        make_identity(nc, ident)
        identf = const.tile([128, 128], F32, tag="identf")
        make_identity(nc, identf)

        wpack = const.tile([128, WCOLS], BF, tag="wpack")
        nc.sync.dma_start(out=wpack[:, :], in_=t["wpack"][:, :])
        bpack = const.tile([128, BCOLS], F32, tag="bpack")
        nc.sync.dma_start(out=bpack[:, :], in_=t["bpack"][:, :])
        gpack = const.tile([128, 6], BF, tag="gpack")
        nc.sync.dma_start(out=gpack[:, :], in_=t["gpack"][:, :])
        wmsg_sb = []
        for wi in range(2):
            mtl = const.tile([NB, E], BF, tag=f"wmsg{wi}")
            nc.sync.dma_start(out=mtl[:, :], in_=t[f"wmsg{wi}"][:, :])
            wmsg_sb.append(mtl)
        epst = const.tile([T, 1], F32, tag="epst")
        nc.vector.memset(epst[:, :], 1e-5)

        def wv(name, nk=None):
            off, n = WOFF[name]
            v = wpack[:, off:off + n]
            if nk is not None:
                v = v.rearrange("p (k e) -> p k e", k=nk)
            return v

        def bv_(name):
            off, n = BOFF[name]
            return bpack[:, off:off + n]

        wev_sb = wv("wev", 16)
        wel_sb = wv("wel")
        wew_sb = wv("wew", 3)
        wsp1a_sb = wv("wsp1a", 2)
        wsp1b_sb = wv("wsp1b", 2)
        wsp2_sb = wv("wsp2")
        w_sb = {nm: wv(nm, 4) for nm in W512}
        wtb2_sb = wv("wtb2")
        wl2_sb = wv("wl2")
        bias = {nm: bv_(nm) for nm in BNAMES2 + BNAMES4 + ["bsum"]}
        bmsg_b = [bias["bmsg0"], bias["bmsg1"]]

        # ---- word embeddings ----
        word_bf = []
        bvec = []
        for wi in range(2):
            gv = gpack[:, wi * 3:(wi + 1) * 3]
            wt = small.tile([128, 2], F32, tag=f"word_f{wi}")
            for eh in range(2):
                p = ps2.tile([128, 1], F32, tag="ps2")
                for kt in range(3):
                    nc.tensor.matmul(p[:, :], wew_sb[:, kt, eh * 128:(eh + 1) * 128],
                                     gv[:, kt:kt + 1], start=(kt == 0), stop=(kt == 2))
                nc.scalar.activation(wt[:, eh:eh + 1], p[:, :], AF.Relu,
                                     bias=bias["b_ew"][:, eh:eh + 1])
            wbf = small.tile([128, 2], BF, tag=f"word_bf{wi}")
            nc.vector.tensor_copy(wbf[:, :], wt[:, :])
            word_bf.append(wbf)
            bv = small.tile([128, 2], F32, tag=f"bvec{wi}")
            for eh in range(2):
                p = ps2.tile([128, 1], F32, tag="ps2")
                for kt in range(2):
                    nc.tensor.matmul(p[:, :], wsp1b_sb[:, kt, eh * 128:(eh + 1) * 128],
                                     wbf[:, kt:kt + 1], start=(kt == 0), stop=(kt == 1))
                nc.vector.scalar_tensor_tensor(
                    bv[:, eh:eh + 1], p[:, :], 1.0, bias["b_sp1"][:, eh:eh + 1],
                    op0=ALU.mult, op1=ALU.add)
            bvec.append(bv)
        relT_bf = small.tile([128, 4], BF, tag="relT")
        nc.vector.tensor_copy(relT_bf[:, 0:2], word_bf[0][:, :])
        nc.vector.tensor_copy(relT_bf[:, 2:4], word_bf[1][:, :])

        # ---- phase A ----
        veT = big.tile([128, 2, M], BF, tag="veT")
        with tc.tile_pool(name="xtp", bufs=2) as xtp:
            for c in range(NCH):
                sl = slice(c * CH, (c + 1) * CH)
                xt_t = xtp.tile([128, KT, CH], BF, tag="xt")
                nc.sync.dma_start(out=xt_t[:, :, :],
                                  in_=t["xt"][:, :, sl].rearrange("k p m -> p k m"))
                for eh in range(2):
                    esl = slice(eh * 128, (eh + 1) * 128)
                    pA = ps.tile([128, CH], F32, tag="ps")
                    for kt in range(16):
                        nc.tensor.matmul(pA[:, :], wev_sb[:, kt, esl], xt_t[:, kt, :],
                                         start=(kt == 0), stop=(kt == 15))
                    pB = ps.tile([128, CH], F32, tag="ps")
                    nc.tensor.matmul(pB[:, :], wel_sb[:, esl], xt_t[:, 16, :],
                                     start=True, stop=True)
                    t1 = work.tile([128, CH], F32, tag="t1")
                    t2 = work.tile([128, CH], F32, tag="t2")
                    nc.scalar.activation(t1[:, :], pA[:, :], AF.Relu,
                                         bias=bias["b_ev"][:, eh:eh + 1])
                    nc.scalar.activation(t2[:, :], pB[:, :], AF.Relu,
                                         bias=bias["b_el"][:, eh:eh + 1])
                    nc.vector.tensor_add(veT[:, eh, sl], t1[:, :], t2[:, :])

        lstmw = ctx.enter_context(tc.tile_pool(name="lstmw", bufs=1))
        wiht_sb = lstmw.tile([128, 4, 4 * H], BF, tag="wiht")
        nc.sync.dma_start(out=wiht_sb[:, :, :],
                          in_=t["wiht"][:, :, :].rearrange("k p g -> p k g"))
        whht_sb = lstmw.tile([128, 4, 4 * H], BF, tag="whht")
        nc.sync.dma_start(out=whht_sb[:, :, :],
                          in_=t["whht"][:, :, :].rearrange("k p g -> p k g"))

        # ---- phase B: spatial attention ----
        featT = []
        msgT = []
        att_out = [t["o_suba"], t["o_obja"]]
        for wi in range(2):
            srow = big.tile([1, M], F32, tag=f"srow{wi}")
            for c in range(NCH):
                sl = slice(c * CH, (c + 1) * CH)
                tanhP = work.tile([128, 2, CH], BF, tag="tanhP")
                for eh in range(2):
                    esl = slice(eh * 128, (eh + 1) * 128)
                    pP = ps.tile([128, CH], F32, tag="ps")
                    for kt in range(2):
                        nc.tensor.matmul(pP[:, :], wsp1a_sb[:, kt, esl],
                                         veT[:, kt, sl], start=(kt == 0), stop=(kt == 1))
                    nc.scalar.activation(tanhP[:, eh, :], pP[:, :], AF.Tanh,
                                         bias=bvec[wi][:, eh:eh + 1])
                pS = ps2.tile([1, CH], F32, tag="ps2")
                for kt in range(2):
                    nc.tensor.matmul(pS[:, :], wsp2_sb[:, kt:kt + 1],
                                     tanhP[:, kt, :], start=(kt == 0), stop=(kt == 1))
                nc.scalar.copy(srow[:, sl], pS[:, :])
            S = small.tile([T, NB], F32, tag="S")
            nc.sync.dma_start(out=S[:, :],
                              in_=srow.rearrange("p (t n) -> p t n", t=T))
            mx = small.tile([T, 1], F32, tag="mx")
            nc.vector.tensor_reduce(out=mx[:, :], in_=S[:, :], axis=AX.X, op=ALU.max)
            nmx = small.tile([T, 1], F32, tag="nmx")
            nc.vector.tensor_scalar_mul(nmx[:, :], mx[:, :], -1.0)
            Ea = small.tile([T, NB], F32, tag="Ea")
            nc.scalar.activation(Ea[:, :], S[:, :], AF.Exp, bias=nmx[:, :])
            sm = small.tile([T, 1], F32, tag="sm")
            nc.vector.tensor_reduce(out=sm[:, :], in_=Ea[:, :], axis=AX.X, op=ALU.add)
            rc = small.tile([T, 1], F32, tag="rc")
            nc.vector.reciprocal(rc[:, :], sm[:, :])
            a_f = small.tile([T, NB], F32, tag="a_f")
            nc.vector.tensor_scalar_mul(a_f[:, :], Ea[:, :], rc[:, :])
            nc.sync.dma_start(out=att_out[wi][:, :], in_=a_f[:, :])
            nc.sync.dma_start(out=t["arow_dram"][wi, :], in_=a_f[:, :])
            a_bf = small.tile([T, NB], BF, tag="a_bf")
            nc.vector.tensor_copy(a_bf[:, :], a_f[:, :])
            paT = ps2.tile([NB, T], BF, tag="ps2")
            nc.tensor.transpose(paT[:, :], a_bf[:, :], ident[:T, :T])
            aT_bf = small.tile([NB, T], BF, tag="aT_bf")
            nc.scalar.copy(aT_bf[:, :], paT[:, :])
            mg = small.tile([128, 2, T], F32, tag=f"msgT{wi}")
            for eh in range(2):
                pm = ps.tile([128, T], F32, tag="ps")
                nc.tensor.matmul(pm[:, :], wmsg_sb[wi][:, eh * 128:(eh + 1) * 128],
                                 aT_bf[:, :], start=True, stop=True)
                nc.scalar.activation(mg[:, eh, :], pm[:, :], AF.Relu,
                                     bias=bmsg_b[wi][:, eh:eh + 1])
            msgT.append(mg)
            ft = small.tile([128, 2, T], F32, tag=f"featT{wi}")
            for c in range(NCH):
                aB = work.tile([128, CH], BF, tag="aB")
                nc.gpsimd.dma_start(out=aB[:, :], in_=bass.AP(
                    tensor=t["arow_dram"][:, :].tensor, offset=wi * M + c * CH,
                    ap=[[0, 128], [1, CH]]))
                for eh in range(2):
                    prod = work.tile([128, SEG, NB], F32, tag="prod")
                    nc.vector.tensor_mul(
                        prod[:, :, :],
                        veT[:, eh, c * CH:(c + 1) * CH].rearrange(
                            "p (s n) -> p s n", s=SEG),
                        aB.rearrange("p (s n) -> p s n", s=SEG))
                    nc.vector.tensor_reduce(
                        out=ft[:, eh, c * SEG:(c + 1) * SEG], in_=prod[:, :, :],
                        axis=AX.X, op=ALU.add)
            featT.append(ft)

        # ---- transformer ----
        ori_bf = small.tile([128, 4, T], BF, tag="ori_bf")
        nc.vector.tensor_add(ori_bf[:, 0:2, :], featT[0][:, :, :], msgT[1][:, :, :])
        nc.vector.tensor_add(ori_bf[:, 2:4, :], featT[1][:, :, :], msgT[0][:, :, :])

        def mm512(wname, rhs_bf, n):
            outs = []
            for mh in range(4):
                p = ps.tile([128, n], F32, tag="ps")
                for kt in range(4):
                    nc.tensor.matmul(p[:, :],
                                     w_sb[wname][:, kt, mh * 128:(mh + 1) * 128],
                                     rhs_bf[:, kt, :], start=(kt == 0), stop=(kt == 3))
                outs.append(p)
            return outs

        xT_f = small.tile([128, 4, T], F32, tag="xT_f")
        xT_bf = small.tile([128, 4, T], BF, tag="xT_bf")
        for mh, p in enumerate(mm512("wtv", ori_bf, T)):
            nc.scalar.activation(xT_f[:, mh, :], p[:, :], AF.Relu,
                                 bias=bias["b_tv"][:, mh:mh + 1])
            nc.vector.tensor_copy(xT_bf[:, mh, :], xT_f[:, mh, :])

        qkv = {}
        for nm, wn, bn in [("q", "wq", "bq"), ("k", "wk", "bk"), ("v", "wv", "bv")]:
            dst = small.tile([128, 4, T], BF, tag=nm + "T")
            for mh, p in enumerate(mm512(wn, xT_bf, T)):
                nc.vector.tensor_scalar_add(dst[:, mh, :], p[:, :],
                                            bias[bn][:, mh:mh + 1])
            qkv[nm] = dst

        aoT_bf = small.tile([128, 4, T], BF, tag="aoT")
        for h in range(NH):
            hh, off = h // 2, 64 * (h % 2)
            pSc = ps2.tile([T, T], F32, tag="ps2")
            nc.tensor.matmul(pSc[:, :], qkv["q"][off:off + 64, hh, :],
                             qkv["k"][off:off + 64, hh, :], start=True, stop=True)
            mx1 = small.tile([T, 1], F32, tag="mx1")
            nc.vector.tensor_reduce(out=mx1[:, :], in_=pSc[:, :], axis=AX.X, op=ALU.max)
            nmx1 = small.tile([T, 1], F32, tag="nmx1")
            nc.vector.tensor_scalar_mul(nmx1[:, :], mx1[:, :], -0.125)
            Eh = small.tile([T, T], F32, tag="Eh")
            nc.scalar.activation(Eh[:, :], pSc[:, :], AF.Exp,
                                 bias=nmx1[:, :], scale=0.125)
            sm1 = small.tile([T, 1], F32, tag="sm1")
            nc.vector.tensor_reduce(out=sm1[:, :], in_=Eh[:, :], axis=AX.X, op=ALU.add)
            rc1 = small.tile([T, 1], F32, tag="rc1")
            nc.vector.reciprocal(rc1[:, :], sm1[:, :])
            Ehn = small.tile([T, T], BF, tag="Ehn")
            nc.vector.tensor_scalar_mul(Ehn[:, :], Eh[:, :], rc1[:, :])
            pET = ps2.tile([T, T], BF, tag="ps2")
            nc.tensor.transpose(pET[:, :], Ehn[:, :], ident[:T, :T])
            EhT = small.tile([T, T], BF, tag="EhT")
            nc.scalar.copy(EhT[:, :], pET[:, :])
            pvn = ps2.tile([T, 64], BF, tag="ps2")
            nc.tensor.transpose(pvn[:, :], qkv["v"][off:off + 64, hh, :],
                                ident[off:off + 64, off:off + 64])
            vnat = small.tile([T, 64], BF, tag="vnat")
            nc.scalar.copy(vnat[:, :], pvn[:, :])
            pao = ps2.tile([64, T], F32, tag="ps2")
            nc.tensor.matmul(pao[:, :], vnat[:, :], EhT[:, :], start=True, stop=True)
            nc.scalar.copy(aoT_bf[off:off + 64, hh, :], pao[:, :])

        x1 = small.tile([128, 4, T], F32, tag="x1")
        for mh, p in enumerate(mm512("wo", aoT_bf, T)):
            tt = work.tile([128, T], F32, tag="ott")
            nc.vector.tensor_scalar_add(tt[:, :], p[:, :], bias["bo"][:, mh:mh + 1])
            nc.vector.tensor_add(x1[:, mh, :], xT_f[:, mh, :], tt[:, :])

        def layer_norm(src_f, gname, bname, out_f, out_bf, also_nat_bf=None):
            xnat = small.tile([T, 4, 128], F32, tag="xnat")
            for mh in range(4):
                pt = ps2.tile([T, 128], F32, tag="ps2")
                nc.tensor.transpose(pt[:, :], src_f[:, mh, :], identf[:, :])
                nc.scalar.copy(xnat[:, mh, :], pt[:, :])
            stats = small.tile([T, 6], F32, tag="stats")
            nc.vector.bn_stats(out=stats[:, :], in_=xnat.rearrange("t a b -> t (a b)"))
            mv = small.tile([T, 2], F32, tag="mv")
            nc.vector.bn_aggr(out=mv[:, :], in_=stats[:, :])
            sq = small.tile([T, 1], F32, tag="sq")
            nc.scalar.activation(sq[:, :], mv[:, 1:2], AF.Sqrt, bias=epst[:, :])
            rstd = small.tile([T, 1], F32, tag="rstd")
            nc.vector.reciprocal(rstd[:, :], sq[:, :])
            nmean = small.tile([T, 1], F32, tag="nmean")
            nc.vector.tensor_scalar_mul(nmean[:, :], mv[:, 0:1], -1.0)
            xn = small.tile([T, 4, 128], F32, tag="xn")
            nc.vector.tensor_scalar(xn[:, :, :], xnat[:, :, :], nmean[:, :],
                                    rstd[:, :], op0=ALU.add, op1=ALU.mult)
            if also_nat_bf is not None:
                nc.vector.tensor_copy(also_nat_bf[:, :, :], xn[:, :, :])
            for mh in range(4):
                pt = ps2.tile([128, T], F32, tag="ps2")
                nc.tensor.transpose(pt[:, :], xn[:, mh, :], identf[:T, :T])
                nc.vector.tensor_scalar(out_f[:, mh, :], pt[:, :],
                                        bias[gname][:, mh:mh + 1],
                                        bias[bname][:, mh:mh + 1],
                                        op0=ALU.mult, op1=ALU.add)
                nc.vector.tensor_copy(out_bf[:, mh, :], out_f[:, mh, :])

        x2 = small.tile([128, 4, T], F32, tag="x2")
        x2_bf = small.tile([128, 4, T], BF, tag="x2bf")
        layer_norm(x1, "g1", "be1", x2, x2_bf)

        f1_bf = small.tile([128, 4, T], BF, tag="f1bf")
        for mh, p in enumerate(mm512("wf1", x2_bf, T)):
            nc.scalar.activation(f1_bf[:, mh, :], p[:, :], AF.Relu,
                                 bias=bias["bf1"][:, mh:mh + 1])
        x3 = small.tile([128, 4, T], F32, tag="x3")
        for mh, p in enumerate(mm512("wf2", f1_bf, T)):
            tt = work.tile([128, T], F32, tag="ftt")
            nc.vector.tensor_scalar_add(tt[:, :], p[:, :], bias["bf2"][:, mh:mh + 1])
            nc.vector.tensor_add(x3[:, mh, :], x2[:, mh, :], tt[:, :])

        withinT = small.tile([128, 4, T], F32, tag="withinT")
        withinT_bf = small.tile([128, 4, T], BF, tag="withinTbf")
        within_nat_bf = small.tile([T, 4, 128], BF, tag="withinnat")
        layer_norm(x3, "g2", "be2", withinT, withinT_bf,
                   also_nat_bf=within_nat_bf)

        # ---- phase D ----
        seg_bf = withinT_bf[:, :, SEG - 1::SEG]

        trT_bf = small.tile([128, 4], BF, tag="trTbf")
        for mh in range(4):
            p = ps2.tile([128, 1], F32, tag="ps2")
            for kt in range(4):
                nc.tensor.matmul(p[:, :], w_sb["wtr"][:, kt, mh * 128:(mh + 1) * 128],
                                 relT_bf[:, kt:kt + 1], start=(kt == 0), stop=(kt == 3))
            nc.scalar.activation(trT_bf[:, mh:mh + 1], p[:, :], AF.Relu,
                                 bias=bias["b_tr"][:, mh:mh + 1])

        ub2 = small.tile([128, 4], F32, tag="ub2")
        for mh in range(4):
            p = ps2.tile([128, 1], F32, tag="ps2")
            for kt in range(4):
                nc.tensor.matmul(p[:, :], w_sb["wtb1b"][:, kt, mh * 128:(mh + 1) * 128],
                                 trT_bf[:, kt:kt + 1], start=(kt == 0), stop=(kt == 3))
            nc.vector.tensor_scalar_add(ub2[:, mh:mh + 1], p[:, :],
                                        bias["b_tb1"][:, mh:mh + 1])

        z2_bf = small.tile([128, 4, NSEG], BF, tag="z2bf")
        for mh in range(4):
            p = ps2.tile([128, NSEG], F32, tag="ps2")
            for kt in range(4):
                nc.tensor.matmul(p[:, :], w_sb["wtb1a"][:, kt, mh * 128:(mh + 1) * 128],
                                 seg_bf[:, kt, :], start=(kt == 0), stop=(kt == 3))
            nc.scalar.activation(z2_bf[:, mh, :], p[:, :], AF.Tanh,
                                 bias=ub2[:, mh:mh + 1])

        def row_softmax(psrc, n, tag):
            row = small.tile([1, n], F32, tag=tag + "r")
            nc.scalar.copy(row[:, :], psrc[:, :])
            m_ = small.tile([1, 1], F32, tag=tag + "m")
            nc.vector.tensor_reduce(out=m_[:, :], in_=row[:, :], axis=AX.X, op=ALU.max)
            nm_ = small.tile([1, 1], F32, tag=tag + "nm")
            nc.vector.tensor_scalar_mul(nm_[:, :], m_[:, :], -1.0)
            e_ = small.tile([1, n], F32, tag=tag + "e")
            nc.scalar.activation(e_[:, :], row[:, :], AF.Exp, bias=nm_[:, :])
            s_ = small.tile([1, 1], F32, tag=tag + "s")
            nc.vector.tensor_reduce(out=s_[:, :], in_=e_[:, :], axis=AX.X, op=ALU.add)
            r_ = small.tile([1, 1], F32, tag=tag + "rc")
            nc.vector.reciprocal(r_[:, :], s_[:, :])
            o_ = small.tile([1, n], F32, tag=tag + "o")
            nc.vector.tensor_scalar_mul(o_[:, :], e_[:, :], r_[:, :])
            return o_

        pb2 = ps2.tile([1, NSEG], F32, tag="ps2")
        for kt in range(4):
            nc.tensor.matmul(pb2[:, :], wtb2_sb[:, kt:kt + 1], z2_bf[:, kt, :],
                             start=(kt == 0), stop=(kt == 3))
        beta2row = row_softmax(pb2, NSEG, "b2")
        nc.sync.dma_start(out=t["o_beta2"][None, :], in_=beta2row[:, :])

        XWT = small.tile([128, 16, NSEG], F32, tag="XWT")
        for gt in range(16):
            p = ps2.tile([128, NSEG], F32, tag="ps2")
            for kt in range(4):
                nc.tensor.matmul(p[:, :], wiht_sb[:, kt, gt * 128:(gt + 1) * 128],
                                 seg_bf[:, kt, :], start=(kt == 0), stop=(kt == 3))
            nc.vector.tensor_scalar_add(XWT[:, gt, :], p[:, :],
                                        bias["bsum"][:, gt:gt + 1])

        lstm = ctx.enter_context(tc.tile_pool(name="lstm", bufs=3))
        h_bf = lstm.tile([128, 4], BF, tag="h_bf")
        c_f = lstm.tile([128, 4], F32, tag="c_f")
        nc.vector.memset(h_bf[:, :], 0.0)
        nc.vector.memset(c_f[:, :], 0.0)
        h_f = None
        for s in range(NSEG):
            pg = ps.tile([128, 16], F32, tag="ps")
            for gt in range(16):
                for kt in range(4):
                    nc.tensor.matmul(pg[:, gt:gt + 1],
                                     whht_sb[:, kt, gt * 128:(gt + 1) * 128],
                                     h_bf[:, kt:kt + 1],
                                     start=(kt == 0), stop=(kt == 3))
            G = lstm.tile([128, 16], F32, tag="G")
            nc.vector.tensor_add(G[:, :], pg[:, :], XWT[:, :, s])
            SGt = lstm.tile([128, 16], F32, tag="SGt")
            nc.scalar.activation(SGt[:, 0:8], G[:, 0:8], AF.Sigmoid)
            nc.scalar.activation(SGt[:, 8:12], G[:, 8:12], AF.Tanh)
            nc.scalar.activation(SGt[:, 12:16], G[:, 12:16], AF.Sigmoid)
            t1 = lstm.tile([128, 4], F32, tag="lt1")
            t2 = lstm.tile([128, 4], F32, tag="lt2")
            nc.vector.tensor_mul(t1[:, :], SGt[:, 4:8], c_f[:, :])
            nc.vector.tensor_mul(t2[:, :], SGt[:, 0:4], SGt[:, 8:12])
            c_f = lstm.tile([128, 4], F32, tag="c_f")
            nc.vector.tensor_add(c_f[:, :], t1[:, :], t2[:, :])
            tct = lstm.tile([128, 4], F32, tag="tct")
            nc.scalar.activation(tct[:, :], c_f[:, :], AF.Tanh)
            h_f = lstm.tile([128, 4], F32, tag="h_f")
            nc.vector.tensor_mul(h_f[:, :], SGt[:, 12:16], tct[:, :])
            h_bf = lstm.tile([128, 4], BF, tag="h_bf")
            nc.vector.tensor_copy(h_bf[:, :], h_f[:, :])
        nc.sync.dma_start(out=t["o_ht"][:, :], in_=h_f[:, :])
        nc.sync.dma_start(out=t["o_ct"][:, :], in_=c_f[:, :])

        ub1 = small.tile([128, 4], F32, tag="ub1")
        for mh in range(4):
            p = ps2.tile([128, 1], F32, tag="ps2")
            for kt in range(4):
                nc.tensor.matmul(p[:, :], w_sb["wl1b"][:, kt, mh * 128:(mh + 1) * 128],
                                 h_bf[:, kt:kt + 1], start=(kt == 0), stop=(kt == 3))
            nc.vector.tensor_scalar_add(ub1[:, mh:mh + 1], p[:, :],
                                        bias["b_l1"][:, mh:mh + 1])
        z1_bf = small.tile([128, 4, T], BF, tag="z1bf")
        for mh in range(4):
            p = ps.tile([128, T], F32, tag="ps")
            for kt in range(4):
                nc.tensor.matmul(p[:, :], w_sb["wl1a"][:, kt, mh * 128:(mh + 1) * 128],
                                 withinT_bf[:, kt, :], start=(kt == 0), stop=(kt == 3))
            nc.scalar.activation(z1_bf[:, mh, :], p[:, :], AF.Tanh,
                                 bias=ub1[:, mh:mh + 1])
        pb1 = ps2.tile([1, T], F32, tag="ps2")
        for kt in range(4):
            nc.tensor.matmul(pb1[:, :], wl2_sb[:, kt:kt + 1], z1_bf[:, kt, :],
                             start=(kt == 0), stop=(kt == 3))
        beta1row = row_softmax(pb1, T, "b1")
        nc.sync.dma_start(out=t["o_beta1"][None, :], in_=beta1row[:, :])

        temp_bf = small.tile([1, T], BF, tag="tempbf")
        b2rep = bass.AP(tensor=beta2row[:, :].tensor, offset=beta2row[:, :].offset,
                        ap=[list(beta2row[:, :].ap[0]), [1, NSEG], [0, SEG]])
        nc.vector.tensor_tensor(
            temp_bf[:, :],
            beta1row.rearrange("p (a b) -> p a b", a=NSEG), b2rep, op=ALU.add)
        ptT = ps2.tile([T, 1], BF, tag="ps2")
        nc.tensor.transpose(ptT[:, :], temp_bf[:, :], ident[:1, :1])
        tempT_bf = small.tile([T, 1], BF, tag="tempTbf")
        nc.scalar.copy(tempT_bf[:, :], ptT[:, :])
        outv = small.tile([128, 4], F32, tag="outv")
        for mh in range(4):
            p = ps2.tile([128, 1], F32, tag="ps2")
            nc.tensor.matmul(p[:, :], within_nat_bf[:, mh, :], tempT_bf[:, :],
                             start=True, stop=True)
            nc.scalar.copy(outv[:, mh:mh + 1], p[:, :])
        nc.sync.dma_start(out=t["o_out"][:, :], in_=outv[:, :])


def _prep_maps(inputs):
    f = {k: np.asarray(v, np.float32) for k, v in inputs.items()}

    def ktp(a, nt):  # [nt*128, n] -> [128, nt*n] partition-major pack
        return np.ascontiguousarray(
            a.reshape(nt, 128, a.shape[1]).transpose(1, 0, 2).reshape(128, -1))

    def pad_rows(a, rows):
        out = np.zeros((rows, a.shape[1]), a.dtype)
        out[: a.shape[0]] = a
        return out

    def pcol(v, nt):
        return np.ascontiguousarray(v.reshape(nt, 128).T)

    wp = np.zeros((128, WCOLS), np.float32)
    wp[:, 0:4096] = ktp(f["W_ev"], 16)
    wp[:, 4096:4352] = pad_rows(f["W_el"], 128)
    wp[:, 4352:5120] = ktp(pad_rows(f["W_ew"], 384), 3)
    wp[:, 5120:5632] = ktp(f["W_sp1"][:E], 2)
    wp[:, 5632:6144] = ktp(f["W_sp1"][E:], 2)
    for nm, src in [("wtv", "W_tv"), ("wq", "Wq"), ("wk", "Wk"), ("wv", "Wv"),
                    ("wo", "Wo"), ("wf1", "Wf1"), ("wf2", "Wf2"),
                    ("wtr", "W_tr")]:
        off, n = WOFF[nm]
        wp[:, off:off + n] = ktp(f[src], 4)
    for nm, arr in [("wtb1a", f["W_tb1"][:H]), ("wtb1b", f["W_tb1"][H:]),
                    ("wl1a", f["W_l1"][:H]), ("wl1b", f["W_l1"][H:])]:
        off, n = WOFF[nm]
        wp[:, off:off + n] = ktp(arr, 4)
    for nm, v, nt in [("wsp2", f["w_sp2"], 2), ("wtb2", f["w_tb2"], 4),
                      ("wl2", f["w_l2"], 4)]:
        off, n = WOFF[nm]
        wp[:, off:off + n] = pcol(v, nt)
    wpack = np.ascontiguousarray(wp.astype(BF16))

    bp = np.zeros((128, BCOLS), np.float32)
    for nm, src, nt in [("b_ev", "b_ev", 2), ("b_el", "b_el", 2),
                        ("b_ew", "b_ew", 2), ("b_sp1", "b_sp1", 2),
                        ("bmsg0", "b_s2o", 2), ("bmsg1", "b_o2s", 2),
                        ("b_tv", "b_tv", 4), ("bq", "bq", 4), ("bk", "bk", 4),
                        ("bv", "bv", 4), ("bo", "bo", 4), ("bf1", "bf1", 4),
                        ("bf2", "bf2", 4), ("b_tr", "b_tr", 4),
                        ("b_tb1", "b_tb1", 4), ("b_l1", "b_l1", 4),
                        ("g1", "g1", 4), ("be1", "be1", 4), ("g2", "g2", 4),
                        ("be2", "be2", 4)]:
        off, n = BOFF[nm]
        bp[:, off:off + n] = pcol(f[src], nt)
    off, n = BOFF["bsum"]
    bp[:, off:off + n] = pcol(f["b_ih"] + f["b_hh"], 16)
    bpack = np.ascontiguousarray(bp)

    shared = {
        "wpack": wpack,
        "bpack": bpack,
        "wmsg0": np.ascontiguousarray(f["W_s2o"].astype(BF16)),
        "wmsg1": np.ascontiguousarray(f["W_o2s"].astype(BF16)),
        "wiht": np.ascontiguousarray(
            f["W_ih"].T.reshape(4, 128, 4 * H).astype(BF16)),
        "whht": np.ascontiguousarray(
            f["W_hh"].T.reshape(4, 128, 4 * H).astype(BF16)),
    }

    in_maps = []
    for b in range(B):
        m = dict(shared)
        x = f["videos"][b].reshape(M, VD)
        xt = np.zeros((KT * 128, M), BF16)
        xt[:VD] = np.ascontiguousarray(x.T).astype(BF16)
        m["xt"] = xt.reshape(KT, 128, M)
        gp = np.zeros((128, 6), np.float32)
        gp[:, 0:3] = pad_rows(f["sub_glove"][b][:, None], 384).reshape(3, 128).T
        gp[:, 3:6] = pad_rows(f["obj_glove"][b][:, None], 384).reshape(3, 128).T
        m["gpack"] = np.ascontiguousarray(gp.astype(BF16))
        in_maps.append(m)
    return in_maps


def _make_runner(nc):
    """Cached jitted SPMD executor mirroring bass2jax.run_bass_via_pjrt."""
    import jax
    import mybir as _  # noqa: F401
    from concourse import bass2jax, mybir as mb
    from jax.sharding import Mesh, PartitionSpec
    from jax.experimental.shard_map import shard_map

    bass2jax.install_neuronx_cc_hook()
    partition_name = nc.partition_id_tensor.name if nc.partition_id_tensor else None
    in_names, out_names, out_avals = [], [], []
    for alloc in nc.m.functions[0].allocations:
        if not isinstance(alloc, mb.MemoryLocationSet):
            continue
        name = alloc.memorylocations[0].name
        if alloc.kind == "ExternalInput":
            if name != partition_name:
                in_names.append(name)
        elif alloc.kind == "ExternalOutput":
            out_names.append(name)
            out_avals.append(jax.core.ShapedArray(
                tuple(alloc.tensor_shape), mb.dt.np(alloc.dtype)))
    n_params = len(in_names)
    all_names = in_names + out_names
    if partition_name is not None:
        all_names.append(partition_name)

    def _body(*args):
        operands = list(args)
        if partition_name is not None:
            operands.append(bass2jax.partition_id_tensor())
        outs = bass2jax._bass_exec_p.bind(
            *operands, out_avals=tuple(out_avals), in_names=tuple(all_names),
            out_names=tuple(out_names), lowering_input_output_aliases=(),
            sim_require_finite=True, sim_require_nnan=True, nc=nc)
        return tuple(outs)

    devices = jax.devices()[:B]
    mesh = Mesh(np.asarray(devices), ("core",))
    n_outs = len(out_names)
    sharded = jax.jit(
        shard_map(_body, mesh=mesh,
                  in_specs=(PartitionSpec("core"),) * (n_params + n_outs),
                  out_specs=(PartitionSpec("core"),) * n_outs,
                  check_rep=False),
        donate_argnums=tuple(range(n_params, n_params + n_outs)),
        keep_unused=True)
    return sharded, in_names, out_names, out_avals


def _run(nc, in_maps):
    import jax

    if "runner" not in _CACHE:
        _CACHE["runner"] = _make_runner(nc)
    sharded, in_names, out_names, out_avals = _CACHE["runner"]
    concat_in = [np.concatenate([np.asarray(m[nm]) for m in in_maps], axis=0)
                 for nm in in_names]
    concat_in = [jax.device_put(a) for a in concat_in]
    for a in concat_in:
        a.block_until_ready()

    def one_call():
        zeros = [np.zeros((B * av.shape[0], *av.shape[1:]), av.dtype)
                 for av in out_avals]
        out = sharded(*concat_in, *zeros)
        for o in out:
            o.block_until_ready()
        return out

    out_arrs = one_call()
    results = [
        {nm: np.asarray(out_arrs[i]).reshape(B, *out_avals[i].shape)[c]
         for i, nm in enumerate(out_names)}
        for c in range(B)
    ]
    timing = None
    if os.environ.get("KERNEL_TRACE") == "1":
        import time as _time

        best = None
        for _ in range(10):
            t0 = _time.perf_counter()
            one_call()
            dt = _time.perf_counter() - t0
            best = dt if best is None else min(best, dt)
        timing = int(best * 1e9)
    return results, timing


def kernel(**inputs):
    global last_exec_time_ns

    if "nc" not in _CACHE:
        _CACHE["nc"] = _build_nc()
    nc = _CACHE["nc"]
    in_maps = _prep_maps(inputs)
    outs, timing = _run(nc, in_maps)
    if timing is not None:
        last_exec_time_ns = timing
    output = np.stack([r["o_out"].T.reshape(H) for r in outs]).astype(np.float32)
    hT = np.stack([r["o_ht"].T.reshape(H) for r in outs]).astype(np.float32)
    cT = np.stack([r["o_ct"].T.reshape(H) for r in outs]).astype(np.float32)
    sub_att = np.stack([r["o_suba"] for r in outs]).astype(np.float32)
    obj_att = np.stack([r["o_obja"] for r in outs]).astype(np.float32)
    beta1 = np.stack([r["o_beta1"] for r in outs]).astype(np.float32)
    beta2 = np.stack([r["o_beta2"] for r in outs]).astype(np.float32)
    return (output, hT, cT, sub_att, obj_att, beta1, beta2)


# revision 16
# speedup vs baseline: 16040.4386x; 2226.0862x over previous
"""AttHierarchicalGround Trainium2 kernel.

Pure data parallel over batch (B=8 -> 1 element per NeuronCore). Host
pre-transposes the video tensor per core to [K=2176(pad), M=4800] bf16 so the
contraction dim lands on SBUF partitions with no on-device transposes of X.
All weights ship packed in one bf16 DRAM tensor (single DMA -> single sem
lane), biases in one f32 tensor. Matmuls run bf16 with fp32 PSUM accumulation;
elementwise/softmax/LN/LSTM state stay fp32.
"""

import os
import sys

import numpy as np

sys.path.insert(0, "/opt/trn_rl_repo")

import ml_dtypes  # noqa: E402

import concourse.bass as bass  # noqa: E402
import concourse.bacc as bacc  # noqa: E402
import concourse.mybir as mybir  # noqa: E402
import concourse.tile as tile  # noqa: E402
from concourse.masks import make_identity  # noqa: E402

BF16 = ml_dtypes.bfloat16
F32 = mybir.dt.float32
BF = mybir.dt.bfloat16
AF = mybir.ActivationFunctionType
ALU = mybir.AluOpType
AX = mybir.AxisListType

B, T, NB, VD, WD = 8, 120, 40, 2053, 300
H, E, NH, HD = 512, 256, 8, 64
SEG, NSEG = 12, 10
M = T * NB
KT = 17
CH = 480
NCH = M // CH

W512 = ["wtv", "wq", "wk", "wv", "wo", "wf1", "wf2", "wtr",
        "wtb1a", "wtb1b", "wl1a", "wl1b"]
# packed bf16 weight columns
WOFF = {"wev": (0, 4096), "wel": (4096, 256), "wew": (4352, 768),
        "wsp1a": (5120, 512), "wsp1b": (5632, 512)}
_off = 6144
for _nm in W512:
    WOFF[_nm] = (_off, 2048)
    _off += 2048
WOFF["wsp2"] = (_off, 2); _off += 2
WOFF["wtb2"] = (_off, 4); _off += 4
WOFF["wl2"] = (_off, 4); _off += 4
WCOLS = _off  # 30730

BNAMES2 = ["b_ev", "b_el", "b_ew", "b_sp1", "bmsg0", "bmsg1"]
BNAMES4 = ["b_tv", "bq", "bk", "bv", "bo", "bf1", "bf2", "b_tr",
           "b_tb1", "b_l1", "g1", "be1", "g2", "be2"]
BOFF = {}
_bo = 0
for _nm in BNAMES2:
    BOFF[_nm] = (_bo, 2)
    _bo += 2
for _nm in BNAMES4:
    BOFF[_nm] = (_bo, 4)
    _bo += 4
BOFF["bsum"] = (_bo, 16)
BCOLS = _bo + 16  # 84

_CACHE = {}
last_exec_time_ns = None


def _build_nc():
    nc = bacc.Bacc(trn_type="TRN2", target_bir_lowering=False)
    t = {}
    t["xt"] = nc.dram_tensor("xt", [KT, 128, M], BF, kind="ExternalInput")
    t["wpack"] = nc.dram_tensor("wpack", [128, WCOLS], BF, kind="ExternalInput")
    t["bpack"] = nc.dram_tensor("bpack", [128, BCOLS], F32, kind="ExternalInput")
    t["gpack"] = nc.dram_tensor("gpack", [128, 6], BF, kind="ExternalInput")
    t["wmsg0"] = nc.dram_tensor("wmsg0", [NB, E], BF, kind="ExternalInput")
    t["wmsg1"] = nc.dram_tensor("wmsg1", [NB, E], BF, kind="ExternalInput")
    t["wiht"] = nc.dram_tensor("wiht", [4, 128, 4 * H], BF, kind="ExternalInput")
    t["whht"] = nc.dram_tensor("whht", [4, 128, 4 * H], BF, kind="ExternalInput")
    t["o_out"] = nc.dram_tensor("o_out", [128, 4], F32, kind="ExternalOutput")
    t["o_ht"] = nc.dram_tensor("o_ht", [128, 4], F32, kind="ExternalOutput")
    t["o_ct"] = nc.dram_tensor("o_ct", [128, 4], F32, kind="ExternalOutput")
    t["o_suba"] = nc.dram_tensor("o_suba", [T, NB], F32, kind="ExternalOutput")
    t["o_obja"] = nc.dram_tensor("o_obja", [T, NB], F32, kind="ExternalOutput")
    t["o_beta1"] = nc.dram_tensor("o_beta1", [T], F32, kind="ExternalOutput")
    t["o_beta2"] = nc.dram_tensor("o_beta2", [NSEG], F32, kind="ExternalOutput")
    t["arow_dram"] = nc.dram_tensor("arow_scratch", [2, M], F32)
    with tile.TileContext(nc) as tc:
        _emit(nc, tc, t)
    nc.compile()
    return nc


def _emit(nc, tc, t):
    from contextlib import ExitStack

    ctx = ExitStack()
    with ctx:
        const = ctx.enter_context(tc.tile_pool(name="const", bufs=1))
        big = ctx.enter_context(tc.tile_pool(name="big", bufs=1))
        work = ctx.enter_context(tc.tile_pool(name="work", bufs=2))
        small = ctx.enter_context(tc.tile_pool(name="small", bufs=1))
        ps = ctx.enter_context(tc.tile_pool(name="ps", bufs=4, space="PSUM"))
        ps2 = ctx.enter_context(tc.tile_pool(name="ps2", bufs=4, space="PSUM"))

        ident = const.tile([128, 128], BF, tag="ident")
        make_identity(nc, ident)
        identf = const.tile([128, 128], F32, tag="identf")
        make_identity(nc, identf)

        wpack = const.tile([128, WCOLS], BF, tag="wpack")
        nc.sync.dma_start(out=wpack[:, :], in_=t["wpack"][:, :])
        bpack = const.tile([128, BCOLS], F32, tag="bpack")
        nc.sync.dma_start(out=bpack[:, :], in_=t["bpack"][:, :])
        gpack = const.tile([128, 6], BF, tag="gpack")
        nc.sync.dma_start(out=gpack[:, :], in_=t["gpack"][:, :])
        wmsg_sb = []
        for wi in range(2):
            mtl = const.tile([NB, E], BF, tag=f"wmsg{wi}")
            nc.sync.dma_start(out=mtl[:, :], in_=t[f"wmsg{wi}"][:, :])
            wmsg_sb.append(mtl)
        epst = const.tile([T, 1], F32, tag="epst")
        nc.vector.memset(epst[:, :], 1e-5)

        def wv(name, nk=None):
            off, n = WOFF[name]
            v = wpack[:, off:off + n]
            if nk is not None:
                v = v.rearrange("p (k e) -> p k e", k=nk)
            return v

        def bv_(name):
            off, n = BOFF[name]
            return bpack[:, off:off + n]

        wev_sb = wv("wev", 16)
        wel_sb = wv("wel")
        wew_sb = wv("wew", 3)
        wsp1a_sb = wv("wsp1a", 2)
        wsp1b_sb = wv("wsp1b", 2)
        wsp2_sb = wv("wsp2")
        w_sb = {nm: wv(nm, 4) for nm in W512}
        wtb2_sb = wv("wtb2")
        wl2_sb = wv("wl2")
        bias = {nm: bv_(nm) for nm in BNAMES2 + BNAMES4 + ["bsum"]}
        bmsg_b = [bias["bmsg0"], bias["bmsg1"]]

        # ---- word embeddings ----
        word_bf = []
        bvec = []
        for wi in range(2):
            gv = gpack[:, wi * 3:(wi + 1) * 3]
            wt = small.tile([128, 2], F32, tag=f"word_f{wi}")
            for eh in range(2):
                p = ps2.tile([128, 1], F32, tag="ps2")
                for kt in range(3):
                    nc.tensor.matmul(p[:, :], wew_sb[:, kt, eh * 128:(eh + 1) * 128],
                                     gv[:, kt:kt + 1], start=(kt == 0), stop=(kt == 2))
                nc.scalar.activation(wt[:, eh:eh + 1], p[:, :], AF.Relu,
                                     bias=bias["b_ew"][:, eh:eh + 1])
            wbf = small.tile([128, 2], BF, tag=f"word_bf{wi}")
            nc.vector.tensor_copy(wbf[:, :], wt[:, :])
            word_bf.append(wbf)
            bv = small.tile([128, 2], F32, tag=f"bvec{wi}")
            for eh in range(2):
                p = ps2.tile([128, 1], F32, tag="ps2")
                for kt in range(2):
                    nc.tensor.matmul(p[:, :], wsp1b_sb[:, kt, eh * 128:(eh + 1) * 128],
                                     wbf[:, kt:kt + 1], start=(kt == 0), stop=(kt == 1))
                nc.vector.scalar_tensor_tensor(
                    bv[:, eh:eh + 1], p[:, :], 1.0, bias["b_sp1"][:, eh:eh + 1],
                    op0=ALU.mult, op1=ALU.add)
            bvec.append(bv)
        relT_bf = small.tile([128, 4], BF, tag="relT")
        nc.vector.tensor_copy(relT_bf[:, 0:2], word_bf[0][:, :])
        nc.vector.tensor_copy(relT_bf[:, 2:4], word_bf[1][:, :])

        # ---- phase A ----
        veT = big.tile([128, 2, M], BF, tag="veT")
        with tc.tile_pool(name="xtp", bufs=2) as xtp:
            for c in range(NCH):
                sl = slice(c * CH, (c + 1) * CH)
                xt_t = xtp.tile([128, KT, CH], BF, tag="xt")
                nc.sync.dma_start(out=xt_t[:, :, :],
                                  in_=t["xt"][:, :, sl].rearrange("k p m -> p k m"))
                for eh in range(2):
                    esl = slice(eh * 128, (eh + 1) * 128)
                    pA = ps.tile([128, CH], F32, tag="ps")
                    for kt in range(16):
                        nc.tensor.matmul(pA[:, :], wev_sb[:, kt, esl], xt_t[:, kt, :],
                                         start=(kt == 0), stop=(kt == 15))
                    pB = ps.tile([128, CH], F32, tag="ps")
                    nc.tensor.matmul(pB[:, :], wel_sb[:, esl], xt_t[:, 16, :],
                                     start=True, stop=True)
                    t1 = work.tile([128, CH], F32, tag="t1")
                    t2 = work.tile([128, CH], F32, tag="t2")
                    nc.scalar.activation(t1[:, :], pA[:, :], AF.Relu,
                                         bias=bias["b_ev"][:, eh:eh + 1])
                    nc.scalar.activation(t2[:, :], pB[:, :], AF.Relu,
                                         bias=bias["b_el"][:, eh:eh + 1])
                    nc.vector.tensor_add(veT[:, eh, sl], t1[:, :], t2[:, :])

        lstmw = ctx.enter_context(tc.tile_pool(name="lstmw", bufs=1))
        wiht_sb = lstmw.tile([128, 4, 4 * H], BF, tag="wiht")
        nc.sync.dma_start(out=wiht_sb[:, :, :],
                          in_=t["wiht"][:, :, :].rearrange("k p g -> p k g"))
        whht_sb = lstmw.tile([128, 4, 4 * H], BF, tag="whht")
        nc.sync.dma_start(out=whht_sb[:, :, :],
                          in_=t["whht"][:, :, :].rearrange("k p g -> p k g"))

        # ---- phase B: spatial attention ----
        featT = []
        msgT = []
        att_out = [t["o_suba"], t["o_obja"]]
        for wi in range(2):
            srow = big.tile([1, M], F32, tag=f"srow{wi}")
            for c in range(NCH):
                sl = slice(c * CH, (c + 1) * CH)
                tanhP = work.tile([128, 2, CH], BF, tag="tanhP")
                for eh in range(2):
                    esl = slice(eh * 128, (eh + 1) * 128)
                    pP = ps.tile([128, CH], F32, tag="ps")
                    for kt in range(2):
                        nc.tensor.matmul(pP[:, :], wsp1a_sb[:, kt, esl],
                                         veT[:, kt, sl], start=(kt == 0), stop=(kt == 1))
                    nc.scalar.activation(tanhP[:, eh, :], pP[:, :], AF.Tanh,
                                         bias=bvec[wi][:, eh:eh + 1])
                pS = ps2.tile([1, CH], F32, tag="ps2")
                for kt in range(2):
                    nc.tensor.matmul(pS[:, :], wsp2_sb[:, kt:kt + 1],
                                     tanhP[:, kt, :], start=(kt == 0), stop=(kt == 1))
                nc.scalar.copy(srow[:, sl], pS[:, :])
            S = small.tile([T, NB], F32, tag="S")
            nc.sync.dma_start(out=S[:, :],
                              in_=srow.rearrange("p (t n) -> p t n", t=T))
            mx = small.tile([T, 1], F32, tag="mx")
            nc.vector.tensor_reduce(out=mx[:, :], in_=S[:, :], axis=AX.X, op=ALU.max)
            nmx = small.tile([T, 1], F32, tag="nmx")
            nc.vector.tensor_scalar_mul(nmx[:, :], mx[:, :], -1.0)
            Ea = small.tile([T, NB], F32, tag="Ea")
            nc.scalar.activation(Ea[:, :], S[:, :], AF.Exp, bias=nmx[:, :])
            sm = small.tile([T, 1], F32, tag="sm")
            nc.vector.tensor_reduce(out=sm[:, :], in_=Ea[:, :], axis=AX.X, op=ALU.add)
            rc = small.tile([T, 1], F32, tag="rc")
            nc.vector.reciprocal(rc[:, :], sm[:, :])
            a_f = small.tile([T, NB], F32, tag="a_f")
            nc.vector.tensor_scalar_mul(a_f[:, :], Ea[:, :], rc[:, :])
            nc.sync.dma_start(out=att_out[wi][:, :], in_=a_f[:, :])
            nc.sync.dma_start(out=t["arow_dram"][wi, :], in_=a_f[:, :])
            a_bf = small.tile([T, NB], BF, tag="a_bf")
            nc.vector.tensor_copy(a_bf[:, :], a_f[:, :])
            paT = ps2.tile([NB, T], BF, tag="ps2")
            nc.tensor.transpose(paT[:, :], a_bf[:, :], ident[:T, :T])
            aT_bf = small.tile([NB, T], BF, tag="aT_bf")
            nc.scalar.copy(aT_bf[:, :], paT[:, :])
            mg = small.tile([128, 2, T], F32, tag=f"msgT{wi}")
            for eh in range(2):
                pm = ps.tile([128, T], F32, tag="ps")
                nc.tensor.matmul(pm[:, :], wmsg_sb[wi][:, eh * 128:(eh + 1) * 128],
                                 aT_bf[:, :], start=True, stop=True)
                nc.scalar.activation(mg[:, eh, :], pm[:, :], AF.Relu,
                                     bias=bmsg_b[wi][:, eh:eh + 1])
            msgT.append(mg)
            ft = small.tile([128, 2, T], F32, tag=f"featT{wi}")
            for c in range(NCH):
                aB = work.tile([128, CH], BF, tag="aB")
                nc.gpsimd.dma_start(out=aB[:, :], in_=bass.AP(
                    tensor=t["arow_dram"][:, :].tensor, offset=wi * M + c * CH,
                    ap=[[0, 128], [1, CH]]))
                for eh in range(2):
                    prod = work.tile([128, SEG, NB], F32, tag="prod")
                    nc.vector.tensor_mul(
                        prod[:, :, :],
                        veT[:, eh, c * CH:(c + 1) * CH].rearrange(
                            "p (s n) -> p s n", s=SEG),
                        aB.rearrange("p (s n) -> p s n", s=SEG))
                    nc.vector.tensor_reduce(
                        out=ft[:, eh, c * SEG:(c + 1) * SEG], in_=prod[:, :, :],
                        axis=AX.X, op=ALU.add)
            featT.append(ft)

        # ---- transformer ----
        ori_bf = small.tile([128, 4, T], BF, tag="ori_bf")
        nc.vector.tensor_add(ori_bf[:, 0:2, :], featT[0][:, :, :], msgT[1][:, :, :])
        nc.vector.tensor_add(ori_bf[:, 2:4, :], featT[1][:, :, :], msgT[0][:, :, :])

        def mm512(wname, rhs_bf, n):
            outs = []
            for mh in range(4):
                p = ps.tile([128, n], F32, tag="ps")
                for kt in range(4):
                    nc.tensor.matmul(p[:, :],
                                     w_sb[wname][:, kt, mh * 128:(mh + 1) * 128],
                                     rhs_bf[:, kt, :], start=(kt == 0), stop=(kt == 3))
                outs.append(p)
            return outs

        xT_f = small.tile([128, 4, T], F32, tag="xT_f")
        xT_bf = small.tile([128, 4, T], BF, tag="xT_bf")
        for mh, p in enumerate(mm512("wtv", ori_bf, T)):
            nc.scalar.activation(xT_f[:, mh, :], p[:, :], AF.Relu,
                                 bias=bias["b_tv"][:, mh:mh + 1])
            nc.vector.tensor_copy(xT_bf[:, mh, :], xT_f[:, mh, :])

        qkv = {}
        for nm, wn, bn in [("q", "wq", "bq"), ("k", "wk", "bk"), ("v", "wv", "bv")]:
            dst = small.tile([128, 4, T], BF, tag=nm + "T")
            for mh, p in enumerate(mm512(wn, xT_bf, T)):
                nc.vector.tensor_scalar_add(dst[:, mh, :], p[:, :],
                                            bias[bn][:, mh:mh + 1])
            qkv[nm] = dst

        aoT_bf = small.tile([128, 4, T], BF, tag="aoT")
        for h in range(NH):
            hh, off = h // 2, 64 * (h % 2)
            pSc = ps2.tile([T, T], F32, tag="ps2")
            nc.tensor.matmul(pSc[:, :], qkv["q"][off:off + 64, hh, :],
                             qkv["k"][off:off + 64, hh, :], start=True, stop=True)
            mx1 = work.tile([T, 1], F32, tag="mx1")
            nc.vector.tensor_reduce(out=mx1[:, :], in_=pSc[:, :], axis=AX.X, op=ALU.max)
            nmx1 = work.tile([T, 1], F32, tag="nmx1")
            nc.vector.tensor_scalar_mul(nmx1[:, :], mx1[:, :], -0.125)
            Eh = work.tile([T, T], F32, tag="Eh")
            nc.scalar.activation(Eh[:, :], pSc[:, :], AF.Exp,
                                 bias=nmx1[:, :], scale=0.125)
            sm1 = work.tile([T, 1], F32, tag="sm1")
            nc.vector.tensor_reduce(out=sm1[:, :], in_=Eh[:, :], axis=AX.X, op=ALU.add)
            rc1 = work.tile([T, 1], F32, tag="rc1")
            nc.vector.reciprocal(rc1[:, :], sm1[:, :])
            Ehn = work.tile([T, T], BF, tag="Ehn")
            nc.vector.tensor_scalar_mul(Ehn[:, :], Eh[:, :], rc1[:, :])
            pET = ps2.tile([T, T], BF, tag="ps2")
            nc.tensor.transpose(pET[:, :], Ehn[:, :], ident[:T, :T])
            EhT = work.tile([T, T], BF, tag="EhT")
            nc.scalar.copy(EhT[:, :], pET[:, :])
            pvn = ps2.tile([T, 64], BF, tag="ps2")
            nc.tensor.transpose(pvn[:, :], qkv["v"][off:off + 64, hh, :],
                                ident[off:off + 64, off:off + 64])
            vnat = work.tile([T, 64], BF, tag="vnat")
            nc.scalar.copy(vnat[:, :], pvn[:, :])
            pao = ps2.tile([64, T], F32, tag="ps2")
            nc.tensor.matmul(pao[:, :], vnat[:, :], EhT[:, :], start=True, stop=True)
            nc.scalar.copy(aoT_bf[off:off + 64, hh, :], pao[:, :])

        x1 = small.tile([128, 4, T], F32, tag="x1")
        for mh, p in enumerate(mm512("wo", aoT_bf, T)):
            tt = work.tile([128, T], F32, tag="ott")
            nc.vector.tensor_scalar_add(tt[:, :], p[:, :], bias["bo"][:, mh:mh + 1])
            nc.vector.tensor_add(x1[:, mh, :], xT_f[:, mh, :], tt[:, :])

        def layer_norm(src_f, gname, bname, out_f, out_bf, also_nat_bf=None):
            xnat = small.tile([T, 4, 128], F32, tag="xnat")
            for mh in range(4):
                pt = ps2.tile([T, 128], F32, tag="ps2")
                nc.tensor.transpose(pt[:, :], src_f[:, mh, :], identf[:, :])
                nc.scalar.copy(xnat[:, mh, :], pt[:, :])
            stats = small.tile([T, 6], F32, tag="stats")
            nc.vector.bn_stats(out=stats[:, :], in_=xnat.rearrange("t a b -> t (a b)"))
            mv = small.tile([T, 2], F32, tag="mv")
            nc.vector.bn_aggr(out=mv[:, :], in_=stats[:, :])
            sq = small.tile([T, 1], F32, tag="sq")
            nc.scalar.activation(sq[:, :], mv[:, 1:2], AF.Sqrt, bias=epst[:, :])
            rstd = small.tile([T, 1], F32, tag="rstd")
            nc.vector.reciprocal(rstd[:, :], sq[:, :])
            nmean = small.tile([T, 1], F32, tag="nmean")
            nc.vector.tensor_scalar_mul(nmean[:, :], mv[:, 0:1], -1.0)
            xn = small.tile([T, 4, 128], F32, tag="xn")
            nc.vector.tensor_scalar(xn[:, :, :], xnat[:, :, :], nmean[:, :],
                                    rstd[:, :], op0=ALU.add, op1=ALU.mult)
            if also_nat_bf is not None:
                nc.vector.tensor_copy(also_nat_bf[:, :, :], xn[:, :, :])
            for mh in range(4):
                pt = ps2.tile([128, T], F32, tag="ps2")
                nc.tensor.transpose(pt[:, :], xn[:, mh, :], identf[:T, :T])
                nc.vector.tensor_scalar(out_f[:, mh, :], pt[:, :],
                                        bias[gname][:, mh:mh + 1],
                                        bias[bname][:, mh:mh + 1],
                                        op0=ALU.mult, op1=ALU.add)
                nc.vector.tensor_copy(out_bf[:, mh, :], out_f[:, mh, :])

        x2 = small.tile([128, 4, T], F32, tag="x2")
        x2_bf = small.tile([128, 4, T], BF, tag="x2bf")
        layer_norm(x1, "g1", "be1", x2, x2_bf)

        f1_bf = small.tile([128, 4, T], BF, tag="f1bf")
        for mh, p in enumerate(mm512("wf1", x2_bf, T)):
            nc.scalar.activation(f1_bf[:, mh, :], p[:, :], AF.Relu,
                                 bias=bias["bf1"][:, mh:mh + 1])
        x3 = small.tile([128, 4, T], F32, tag="x3")
        for mh, p in enumerate(mm512("wf2", f1_bf, T)):
            tt = work.tile([128, T], F32, tag="ftt")
            nc.vector.tensor_scalar_add(tt[:, :], p[:, :], bias["bf2"][:, mh:mh + 1])
            nc.vector.tensor_add(x3[:, mh, :], x2[:, mh, :], tt[:, :])

        withinT = small.tile([128, 4, T], F32, tag="withinT")
        withinT_bf = small.tile([128, 4, T], BF, tag="withinTbf")
        within_nat_bf = small.tile([T, 4, 128], BF, tag="withinnat")
        layer_norm(x3, "g2", "be2", withinT, withinT_bf,
                   also_nat_bf=within_nat_bf)

        # ---- phase D ----
        seg_bf = withinT_bf[:, :, SEG - 1::SEG]

        trT_bf = small.tile([128, 4], BF, tag="trTbf")
        for mh in range(4):
            p = ps2.tile([128, 1], F32, tag="ps2")
            for kt in range(4):
                nc.tensor.matmul(p[:, :], w_sb["wtr"][:, kt, mh * 128:(mh + 1) * 128],
                                 relT_bf[:, kt:kt + 1], start=(kt == 0), stop=(kt == 3))
            nc.scalar.activation(trT_bf[:, mh:mh + 1], p[:, :], AF.Relu,
                                 bias=bias["b_tr"][:, mh:mh + 1])

        ub2 = small.tile([128, 4], F32, tag="ub2")
        for mh in range(4):
            p = ps2.tile([128, 1], F32, tag="ps2")
            for kt in range(4):
                nc.tensor.matmul(p[:, :], w_sb["wtb1b"][:, kt, mh * 128:(mh + 1) * 128],
                                 trT_bf[:, kt:kt + 1], start=(kt == 0), stop=(kt == 3))
            nc.vector.tensor_scalar_add(ub2[:, mh:mh + 1], p[:, :],
                                        bias["b_tb1"][:, mh:mh + 1])

        z2_bf = small.tile([128, 4, NSEG], BF, tag="z2bf")
        for mh in range(4):
            p = ps2.tile([128, NSEG], F32, tag="ps2")
            for kt in range(4):
                nc.tensor.matmul(p[:, :], w_sb["wtb1a"][:, kt, mh * 128:(mh + 1) * 128],
                                 seg_bf[:, kt, :], start=(kt == 0), stop=(kt == 3))
            nc.scalar.activation(z2_bf[:, mh, :], p[:, :], AF.Tanh,
                                 bias=ub2[:, mh:mh + 1])

        def row_softmax(psrc, n, tag):
            row = small.tile([1, n], F32, tag=tag + "r")
            nc.scalar.copy(row[:, :], psrc[:, :])
            m_ = small.tile([1, 1], F32, tag=tag + "m")
            nc.vector.tensor_reduce(out=m_[:, :], in_=row[:, :], axis=AX.X, op=ALU.max)
            nm_ = small.tile([1, 1], F32, tag=tag + "nm")
            nc.vector.tensor_scalar_mul(nm_[:, :], m_[:, :], -1.0)
            e_ = small.tile([1, n], F32, tag=tag + "e")
            nc.scalar.activation(e_[:, :], row[:, :], AF.Exp, bias=nm_[:, :])
            s_ = small.tile([1, 1], F32, tag=tag + "s")
            nc.vector.tensor_reduce(out=s_[:, :], in_=e_[:, :], axis=AX.X, op=ALU.add)
            r_ = small.tile([1, 1], F32, tag=tag + "rc")
            nc.vector.reciprocal(r_[:, :], s_[:, :])
            o_ = small.tile([1, n], F32, tag=tag + "o")
            nc.vector.tensor_scalar_mul(o_[:, :], e_[:, :], r_[:, :])
            return o_

        pb2 = ps2.tile([1, NSEG], F32, tag="ps2")
        for kt in range(4):
            nc.tensor.matmul(pb2[:, :], wtb2_sb[:, kt:kt + 1], z2_bf[:, kt, :],
                             start=(kt == 0), stop=(kt == 3))
        beta2row = row_softmax(pb2, NSEG, "b2")
        nc.sync.dma_start(out=t["o_beta2"][None, :], in_=beta2row[:, :])

        XWT = small.tile([128, 16, NSEG], F32, tag="XWT")
        for gt in range(16):
            p = ps2.tile([128, NSEG], F32, tag="ps2")
            for kt in range(4):
                nc.tensor.matmul(p[:, :], wiht_sb[:, kt, gt * 128:(gt + 1) * 128],
                                 seg_bf[:, kt, :], start=(kt == 0), stop=(kt == 3))
            nc.vector.tensor_scalar_add(XWT[:, gt, :], p[:, :],
                                        bias["bsum"][:, gt:gt + 1])

        lstm = ctx.enter_context(tc.tile_pool(name="lstm", bufs=3))
        h_bf = lstm.tile([128, 4], BF, tag="h_bf")
        c_f = lstm.tile([128, 4], F32, tag="c_f")
        nc.vector.memset(h_bf[:, :], 0.0)
        nc.vector.memset(c_f[:, :], 0.0)
        h_f = None
        for s in range(NSEG):
            pg = ps.tile([128, 16], F32, tag="ps")
            for gt in range(16):
                for kt in range(4):
                    nc.tensor.matmul(pg[:, gt:gt + 1],
                                     whht_sb[:, kt, gt * 128:(gt + 1) * 128],
                                     h_bf[:, kt:kt + 1],
                                     start=(kt == 0), stop=(kt == 3))
            G = lstm.tile([128, 16], F32, tag="G")
            nc.vector.tensor_add(G[:, :], pg[:, :], XWT[:, :, s])
            SGt = lstm.tile([128, 16], F32, tag="SGt")
            nc.scalar.activation(SGt[:, 0:8], G[:, 0:8], AF.Sigmoid)
            nc.scalar.activation(SGt[:, 8:12], G[:, 8:12], AF.Tanh)
            nc.scalar.activation(SGt[:, 12:16], G[:, 12:16], AF.Sigmoid)
            t1 = lstm.tile([128, 4], F32, tag="lt1")
            t2 = lstm.tile([128, 4], F32, tag="lt2")
            nc.vector.tensor_mul(t1[:, :], SGt[:, 4:8], c_f[:, :])
            nc.vector.tensor_mul(t2[:, :], SGt[:, 0:4], SGt[:, 8:12])
            c_f = lstm.tile([128, 4], F32, tag="c_f")
            nc.vector.tensor_add(c_f[:, :], t1[:, :], t2[:, :])
            tct = lstm.tile([128, 4], F32, tag="tct")
            nc.scalar.activation(tct[:, :], c_f[:, :], AF.Tanh)
            h_f = lstm.tile([128, 4], F32, tag="h_f")
            nc.vector.tensor_mul(h_f[:, :], SGt[:, 12:16], tct[:, :])
            h_bf = lstm.tile([128, 4], BF, tag="h_bf")
            nc.vector.tensor_copy(h_bf[:, :], h_f[:, :])
        nc.sync.dma_start(out=t["o_ht"][:, :], in_=h_f[:, :])
        nc.sync.dma_start(out=t["o_ct"][:, :], in_=c_f[:, :])

        ub1 = small.tile([128, 4], F32, tag="ub1")
        for mh in range(4):
            p = ps2.tile([128, 1], F32, tag="ps2")
            for kt in range(4):
                nc.tensor.matmul(p[:, :], w_sb["wl1b"][:, kt, mh * 128:(mh + 1) * 128],
                                 h_bf[:, kt:kt + 1], start=(kt == 0), stop=(kt == 3))
            nc.vector.tensor_scalar_add(ub1[:, mh:mh + 1], p[:, :],
                                        bias["b_l1"][:, mh:mh + 1])
        z1_bf = small.tile([128, 4, T], BF, tag="z1bf")
        for mh in range(4):
            p = ps.tile([128, T], F32, tag="ps")
            for kt in range(4):
                nc.tensor.matmul(p[:, :], w_sb["wl1a"][:, kt, mh * 128:(mh + 1) * 128],
                                 withinT_bf[:, kt, :], start=(kt == 0), stop=(kt == 3))
            nc.scalar.activation(z1_bf[:, mh, :], p[:, :], AF.Tanh,
                                 bias=ub1[:, mh:mh + 1])
        pb1 = ps2.tile([1, T], F32, tag="ps2")
        for kt in range(4):
            nc.tensor.matmul(pb1[:, :], wl2_sb[:, kt:kt + 1], z1_bf[:, kt, :],
                             start=(kt == 0), stop=(kt == 3))
        beta1row = row_softmax(pb1, T, "b1")
        nc.sync.dma_start(out=t["o_beta1"][None, :], in_=beta1row[:, :])

        temp_bf = small.tile([1, T], BF, tag="tempbf")
        b2rep = bass.AP(tensor=beta2row[:, :].tensor, offset=beta2row[:, :].offset,
                        ap=[list(beta2row[:, :].ap[0]), [1, NSEG], [0, SEG]])
        nc.vector.tensor_tensor(
            temp_bf[:, :],
            beta1row.rearrange("p (a b) -> p a b", a=NSEG), b2rep, op=ALU.add)
        ptT = ps2.tile([T, 1], BF, tag="ps2")
        nc.tensor.transpose(ptT[:, :], temp_bf[:, :], ident[:1, :1])
        tempT_bf = small.tile([T, 1], BF, tag="tempTbf")
        nc.scalar.copy(tempT_bf[:, :], ptT[:, :])
        outv = small.tile([128, 4], F32, tag="outv")
        for mh in range(4):
            p = ps2.tile([128, 1], F32, tag="ps2")
            nc.tensor.matmul(p[:, :], within_nat_bf[:, mh, :], tempT_bf[:, :],
                             start=True, stop=True)
            nc.scalar.copy(outv[:, mh:mh + 1], p[:, :])
        nc.sync.dma_start(out=t["o_out"][:, :], in_=outv[:, :])


def _prep_maps(inputs):
    f = {k: np.asarray(v, np.float32) for k, v in inputs.items()}

    def ktp(a, nt):  # [nt*128, n] -> [128, nt*n] partition-major pack
        return np.ascontiguousarray(
            a.reshape(nt, 128, a.shape[1]).transpose(1, 0, 2).reshape(128, -1))

    def pad_rows(a, rows):
        out = np.zeros((rows, a.shape[1]), a.dtype)
        out[: a.shape[0]] = a
        return out

    def pcol(v, nt):
        return np.ascontiguousarray(v.reshape(nt, 128).T)

    wp = np.zeros((128, WCOLS), np.float32)
    wp[:, 0:4096] = ktp(f["W_ev"], 16)
    wp[:, 4096:4352] = pad_rows(f["W_el"], 128)
    wp[:, 4352:5120] = ktp(pad_rows(f["W_ew"], 384), 3)
    wp[:, 5120:5632] = ktp(f["W_sp1"][:E], 2)
    wp[:, 5632:6144] = ktp(f["W_sp1"][E:], 2)
    for nm, src in [("wtv", "W_tv"), ("wq", "Wq"), ("wk", "Wk"), ("wv", "Wv"),
                    ("wo", "Wo"), ("wf1", "Wf1"), ("wf2", "Wf2"),
                    ("wtr", "W_tr")]:
        off, n = WOFF[nm]
        wp[:, off:off + n] = ktp(f[src], 4)
    for nm, arr in [("wtb1a", f["W_tb1"][:H]), ("wtb1b", f["W_tb1"][H:]),
                    ("wl1a", f["W_l1"][:H]), ("wl1b", f["W_l1"][H:])]:
        off, n = WOFF[nm]
        wp[:, off:off + n] = ktp(arr, 4)
    for nm, v, nt in [("wsp2", f["w_sp2"], 2), ("wtb2", f["w_tb2"], 4),
                      ("wl2", f["w_l2"], 4)]:
        off, n = WOFF[nm]
        wp[:, off:off + n] = pcol(v, nt)
    wpack = np.ascontiguousarray(wp.astype(BF16))

    bp = np.zeros((128, BCOLS), np.float32)
    for nm, src, nt in [("b_ev", "b_ev", 2), ("b_el", "b_el", 2),
                        ("b_ew", "b_ew", 2), ("b_sp1", "b_sp1", 2),
                        ("bmsg0", "b_s2o", 2), ("bmsg1", "b_o2s", 2),
                        ("b_tv", "b_tv", 4), ("bq", "bq", 4), ("bk", "bk", 4),
                        ("bv", "bv", 4), ("bo", "bo", 4), ("bf1", "bf1", 4),
                        ("bf2", "bf2", 4), ("b_tr", "b_tr", 4),
                        ("b_tb1", "b_tb1", 4), ("b_l1", "b_l1", 4),
                        ("g1", "g1", 4), ("be1", "be1", 4), ("g2", "g2", 4),
                        ("be2", "be2", 4)]:
        off, n = BOFF[nm]
        bp[:, off:off + n] = pcol(f[src], nt)
    off, n = BOFF["bsum"]
    bp[:, off:off + n] = pcol(f["b_ih"] + f["b_hh"], 16)
    bpack = np.ascontiguousarray(bp)

    shared = {
        "wpack": wpack,
        "bpack": bpack,
        "wmsg0": np.ascontiguousarray(f["W_s2o"].astype(BF16)),
        "wmsg1": np.ascontiguousarray(f["W_o2s"].astype(BF16)),
        "wiht": np.ascontiguousarray(
            f["W_ih"].T.reshape(4, 128, 4 * H).astype(BF16)),
        "whht": np.ascontiguousarray(
            f["W_hh"].T.reshape(4, 128, 4 * H).astype(BF16)),
    }

    in_maps = []
    for b in range(B):
        m = dict(shared)
        x = f["videos"][b].reshape(M, VD)
        xt = np.zeros((KT * 128, M), BF16)
        xt[:VD] = np.ascontiguousarray(x.T).astype(BF16)
        m["xt"] = xt.reshape(KT, 128, M)
        gp = np.zeros((128, 6), np.float32)
        gp[:, 0:3] = pad_rows(f["sub_glove"][b][:, None], 384).reshape(3, 128).T
        gp[:, 3:6] = pad_rows(f["obj_glove"][b][:, None], 384).reshape(3, 128).T
        m["gpack"] = np.ascontiguousarray(gp.astype(BF16))
        in_maps.append(m)
    return in_maps


def _make_runner(nc):
    """Cached jitted SPMD executor mirroring bass2jax.run_bass_via_pjrt."""
    import jax
    import mybir as _  # noqa: F401
    from concourse import bass2jax, mybir as mb
    from jax.sharding import Mesh, PartitionSpec
    from jax.experimental.shard_map import shard_map

    bass2jax.install_neuronx_cc_hook()
    partition_name = nc.partition_id_tensor.name if nc.partition_id_tensor else None
    in_names, out_names, out_avals = [], [], []
    for alloc in nc.m.functions[0].allocations:
        if not isinstance(alloc, mb.MemoryLocationSet):
            continue
        name = alloc.memorylocations[0].name
        if alloc.kind == "ExternalInput":
            if name != partition_name:
                in_names.append(name)
        elif alloc.kind == "ExternalOutput":
            out_names.append(name)
            out_avals.append(jax.core.ShapedArray(
                tuple(alloc.tensor_shape), mb.dt.np(alloc.dtype)))
    n_params = len(in_names)
    all_names = in_names + out_names
    if partition_name is not None:
        all_names.append(partition_name)

    def _body(*args):
        operands = list(args)
        if partition_name is not None:
            operands.append(bass2jax.partition_id_tensor())
        outs = bass2jax._bass_exec_p.bind(
            *operands, out_avals=tuple(out_avals), in_names=tuple(all_names),
            out_names=tuple(out_names), lowering_input_output_aliases=(),
            sim_require_finite=True, sim_require_nnan=True, nc=nc)
        return tuple(outs)

    devices = jax.devices()[:B]
    mesh = Mesh(np.asarray(devices), ("core",))
    n_outs = len(out_names)
    sharded = jax.jit(
        shard_map(_body, mesh=mesh,
                  in_specs=(PartitionSpec("core"),) * (n_params + n_outs),
                  out_specs=(PartitionSpec("core"),) * n_outs,
                  check_rep=False),
        donate_argnums=tuple(range(n_params, n_params + n_outs)),
        keep_unused=True)
    return sharded, in_names, out_names, out_avals


def _run(nc, in_maps):
    import jax

    if "runner" not in _CACHE:
        _CACHE["runner"] = _make_runner(nc)
    sharded, in_names, out_names, out_avals = _CACHE["runner"]
    concat_in = [np.concatenate([np.asarray(m[nm]) for m in in_maps], axis=0)
                 for nm in in_names]
    concat_in = [jax.device_put(a) for a in concat_in]
    for a in concat_in:
        a.block_until_ready()

    def one_call():
        zeros = [np.zeros((B * av.shape[0], *av.shape[1:]), av.dtype)
                 for av in out_avals]
        out = sharded(*concat_in, *zeros)
        for o in out:
            o.block_until_ready()
        return out

    out_arrs = one_call()
    results = [
        {nm: np.asarray(out_arrs[i]).reshape(B, *out_avals[i].shape)[c]
         for i, nm in enumerate(out_names)}
        for c in range(B)
    ]
    timing = None
    if os.environ.get("KERNEL_TRACE") == "1":
        import time as _time

        best = None
        for _ in range(10):
            t0 = _time.perf_counter()
            one_call()
            dt = _time.perf_counter() - t0
            best = dt if best is None else min(best, dt)
        timing = int(best * 1e9)
    return results, timing


def kernel(**inputs):
    global last_exec_time_ns

    if "nc" not in _CACHE:
        _CACHE["nc"] = _build_nc()
    nc = _CACHE["nc"]
    in_maps = _prep_maps(inputs)
    outs, timing = _run(nc, in_maps)
    if timing is not None:
        last_exec_time_ns = timing
    output = np.stack([r["o_out"].T.reshape(H) for r in outs]).astype(np.float32)
    hT = np.stack([r["o_ht"].T.reshape(H) for r in outs]).astype(np.float32)
    cT = np.stack([r["o_ct"].T.reshape(H) for r in outs]).astype(np.float32)
    sub_att = np.stack([r["o_suba"] for r in outs]).astype(np.float32)
    obj_att = np.stack([r["o_obja"] for r in outs]).astype(np.float32)
    beta1 = np.stack([r["o_beta1"] for r in outs]).astype(np.float32)
    beta2 = np.stack([r["o_beta2"] for r in outs]).astype(np.float32)
    return (output, hT, cT, sub_att, obj_att, beta1, beta2)
